# revision 15
# baseline (speedup 1.0000x reference)
"""Two-layer GAT on 8 Trainium2 cores via Bass/Tile — fused single launch.

Strategy (dst-node graph partition, per the sharding hint):
- Nodes are split into 8 contiguous ranges (6250 per core); every edge is
  owned by the core that owns its dst node.
- Single launch per core:
  * Phase A1 (redundant on every core): hx1 = x @ [W1p | W1@Asrc] written as
    fp16 tables with 768B rows [h (c,h-interleaved) 256 | a_src.h 8 | pad],
    split lo/hi at row 32512 so dma_gather's int16 indices stay in range;
    a_dst.h goes to a compact ad1 [npad1, 8] side table. Sentinel rows have
    a_src = -30000 (dummy edges gather them; exp == 0).
  * Phase B1 (layer-1 edge aggregation, per 128-dst-node group):
    two dma_gathers (lo/hi) fetch all edge payload rows [128, EBT, 384].
    The group's a_dst block adg [128, 8] comes from one plain strided DMA
    (dst nodes of a group are contiguous); per-edge a_dst is M_T @ adg on
    the PE, where M_T (and M for the scatter) are built by two batched
    is_equal ops from the dst-offset table. Batched DVE/Act ops compute
    w = exp(lrelu(as+ad) - 6) and G = [h*w | w]; EBT PE matmuls accumulate
    U += M_j.T @ G_j in PSUM. Epilogue: h1 = elu(U/(sum)+b1), transposed on
    PE and folded through W2 immediately: hx2 row = [h1@W2 | a2s.h1 | a2d.h1]
    into an SBUF shard table (the layer-2 dense phase is fused in here).
  * AllGather (on-device collective) of the per-core hx2 shard [6272, 36]
    -> [8, 6272, 36] Shared DRAM, then expanded on-device into two padded
    256B-row tables (ranks 0-3 / 4-7) for int16 dma_gather indexing.
  * Phase B2: per-group aggregation as B1 (adg2 is already in SBUF from the
    fold); writes y rows [6272, 32] f32.
- Host: slices the 8 y shards to 6250 rows each and concatenates.
"""

import sys
for _p in ("/opt/trn_rl_repo",):
    if _p not in sys.path:
        sys.path.append(_p)

import math
import numpy as np

import concourse.bass as bass
import concourse.mybir as mybir
import concourse.tile as tile
from concourse import bacc
from concourse.masks import make_identity

F32 = mybir.dt.float32
F16 = mybir.dt.float16
I16 = mybir.dt.int16

N_CORES = 8
D = 256
HC = 256
H = 8
CH = 32
OUT = 32
P = 128
TRW1 = 384       # hx1 table row width (fp16) = 768B (dma_gather granularity)
RW1 = 272        # written row prefix: 256 h + 8 as + 8 ad
PAY1 = 264
TRW2 = 128       # hx2 padded table row width (fp16) = 256B
RW2 = 36         # compact hx2 row: 32 h2 + 1 as2 + 1 ad2 + 2 pad
PAY2 = 33
SHIFT1 = 6.0
SENT_AS = -30000.0
LOSPLIT1 = 32512  # 254 * 128


def make_cfg(n_valid, n_cores=N_CORES):
    npc = n_valid // n_cores
    assert npc * n_cores == n_valid
    ngroups = math.ceil(npc / P)
    npc_pad = ngroups * P
    assert npc_pad > npc, "need a pad row for the layer-2 sentinel"
    nt1 = math.ceil(n_valid / P)
    npad1 = nt1 * P
    losplit = min(LOSPLIT1, (nt1 // 2) * P)
    lo_rows = losplit + P             # + sentinel row block
    hi_rows = npad1 - losplit
    assert losplit <= 32767 and hi_rows <= 32767
    half = (n_cores // 2) * npc_pad   # L2 lo/hi split (remapped rows)
    assert half <= 32767 and n_cores * npc_pad - half <= 32767
    return dict(
        n_valid=n_valid, n_cores=n_cores, npc=npc, ngroups=ngroups,
        npc_pad=npc_pad, nt1=nt1, npad1=npad1,
        losplit=losplit, lo_rows=lo_rows, hi_rows=hi_rows,
        lo_sent=losplit, hi_sent=n_valid - losplit,
        half2=half, sent2=npc,
    )


def fold_weights(W1, a_src1, a_dst1, W2, a_src2, a_dst2):
    W1 = np.asarray(W1, np.float32)
    a_src1 = np.asarray(a_src1, np.float32)
    a_dst1 = np.asarray(a_dst1, np.float32)
    A_src = np.zeros((HC, H), np.float32)
    A_dst = np.zeros((HC, H), np.float32)
    for h in range(H):
        A_src[h * CH:(h + 1) * CH, h] = a_src1[h]
        A_dst[h * CH:(h + 1) * CH, h] = a_dst1[h]
    # interleave the h-block columns to (c, h) order so DVE broadcast muls
    # on-device have a packed last dim
    perm = (np.arange(CH)[:, None] + np.arange(H)[None, :] * CH).reshape(-1)
    W1ext = np.concatenate([W1[:, perm], W1 @ A_src, W1 @ A_dst], axis=1)
    W2 = np.asarray(W2, np.float32)
    W2e = np.concatenate(
        [W2, W2 @ np.asarray(a_src2, np.float32).T,
         W2 @ np.asarray(a_dst2, np.float32).T,
         np.zeros((HC, RW2 - PAY2 - 1), np.float32)], axis=1)   # [256, 36]
    # rows of W2e must match the (c,h)-permuted h1 produced by the PE
    # transposes: contraction index kk*128 + (c - 16*kk)*8 + h <-> row h*32+c
    rperm = np.empty(HC, np.int64)
    i = 0
    for kk in range(2):
        for cl in range(16):
            for h in range(H):
                rperm[i] = h * CH + kk * 16 + cl
                i += 1
    W2ext = W2e[rperm]
    return W1ext.astype(np.float16), W2ext.astype(np.float16)


def _wrap16(arr):
    # dma_gather index layout: ordinal i -> [i % 16, i // 16], x8 rows
    n = arr.size
    return np.tile(arr.reshape(n // 16, 16).T, (8, 1)).astype(np.int16)


def _bucket_tables(src_rows, dst_local, grp, ngroups, losplit, lo_sent,
                   hi_sent, eb_lo, eb_hi):
    """Bucket one core's edges into per-group lo/hi blocks.

    src_rows: table row index per edge; dst_local: dst offset within group
    [0,128); grp: group id per edge. Returns (lo_idx, hi_idx, dstl) with
    lo_idx/hi_idx int16-wrapped [128, ngroups*eb*8] and dstl f16
    [128, ngroups*(eb_lo+eb_hi)].
    """
    islo_all = src_rows < losplit
    ebt = eb_lo + eb_hi
    lo_a = np.full((ngroups, eb_lo * P), lo_sent, np.int64)
    hi_a = np.full((ngroups, eb_hi * P), hi_sent, np.int64)
    dste_a = np.zeros((ngroups, ebt * P), np.int64)
    dstl = np.zeros((P, ngroups * ebt), np.float16)
    order = np.argsort(grp, kind="stable")
    src_rows, dst_local, grp, islo_all = (src_rows[order], dst_local[order],
                                          grp[order], islo_all[order])
    gstart = np.searchsorted(grp, np.arange(ngroups + 1))
    for g in range(ngroups):
        a, b = gstart[g], gstart[g + 1]
        sl, dl, lo = src_rows[a:b], dst_local[a:b], islo_all[a:b]
        nlo = int(lo.sum())
        nhi = (b - a) - nlo
        assert nlo <= eb_lo * P and nhi <= eb_hi * P
        lo_a[g, :nlo] = sl[lo]
        hi_a[g, :nhi] = sl[~lo] - losplit
        ilo = np.arange(nlo)
        dstl[ilo % P, g * ebt + ilo // P] = dl[lo].astype(np.float16)
        dste_a[g, :nlo] = g * P + dl[lo]
        ihi = np.arange(nhi)
        dstl[ihi % P, g * ebt + eb_lo + ihi // P] = dl[~lo].astype(np.float16)
        dste_a[g, eb_lo * P:eb_lo * P + nhi] = g * P + dl[~lo]
    lo_idx = np.concatenate([_wrap16(lo_a[g]) for g in range(ngroups)], axis=1)
    hi_idx = np.concatenate([_wrap16(hi_a[g]) for g in range(ngroups)], axis=1)
    dste = np.concatenate([_wrap16(dste_a[g]) for g in range(ngroups)], axis=1)
    return lo_idx, hi_idx, dstl, dste


def build_edge_tables(src, dst, cfg, ebs=None):
    """Per-core gather tables for both layers. ebs = (eb1_lo, eb1_hi,
    eb2_lo, eb2_hi) or None to size from the data."""
    npc, ngroups = cfg["npc"], cfg["ngroups"]
    npc_pad = cfg["npc_pad"]
    losplit, lo_sent, hi_sent = cfg["losplit"], cfg["lo_sent"], cfg["hi_sent"]
    half2, sent2 = cfg["half2"], cfg["sent2"]
    n_cores = cfg["n_cores"]

    src = np.asarray(src, np.int64)
    dst = np.asarray(dst, np.int64)
    core = dst // npc

    per_core = []
    for k in range(n_cores):
        m = core == k
        s_k = src[m]
        d_k = dst[m] - k * npc
        g_k = d_k // P
        dl_k = d_k - g_k * P
        s2_k = (s_k // npc) * npc_pad + (s_k % npc)   # remapped L2 rows
        per_core.append((s_k, s2_k, dl_k, g_k))

    if ebs is not None:
        e1lo, e1hi, e2lo, e2hi = ebs
    else:
        e1lo = e1hi = e2lo = e2hi = 1
        for s_k, s2_k, dl_k, g_k in per_core:
            islo1 = s_k < losplit
            c = np.bincount(g_k[islo1], minlength=ngroups).max()
            e1lo = max(e1lo, math.ceil(c / P))
            c = np.bincount(g_k[~islo1], minlength=ngroups).max()
            e1hi = max(e1hi, math.ceil(c / P))
            islo2 = s2_k < half2
            c = np.bincount(g_k[islo2], minlength=ngroups).max()
            e2lo = max(e2lo, math.ceil(c / P))
            c = np.bincount(g_k[~islo2], minlength=ngroups).max()
            e2hi = max(e2hi, math.ceil(c / P))

    tables = []
    for s_k, s2_k, dl_k, g_k in per_core:
        lo1, hi1, dstl1, dste1 = _bucket_tables(
            s_k, dl_k, g_k, ngroups, losplit, lo_sent, hi_sent, e1lo, e1hi)
        # L2 sentinel: every rank's local pad row `sent2` has as2=-30000;
        # lo dummies -> rank0's (row sent2), hi dummies -> rank halfc's
        # (local row sent2 within the hi table)
        lo2, hi2, dstl2, dste2 = _bucket_tables(
            s2_k, dl_k, g_k, ngroups, half2, sent2, sent2, e2lo, e2hi)
        tables.append(dict(lo1=lo1, hi1=hi1, dstl1=dstl1, dste1=dste1,
                           lo2=lo2, hi2=hi2, dstl2=dstl2, dste2=dste2))
    npad1 = cfg["npad1"]
    for k, t in enumerate(tables):
        ni = (k * npc + np.arange(P)[:, None]
              + P * np.arange(ngroups)[None, :])
        t["nodeidx"] = np.minimum(ni, npad1 - 1).astype(np.int32)
    return tables, (e1lo, e1hi, e2lo, e2hi)


# --------------------------------------------------------------------------
# fused launch
# --------------------------------------------------------------------------

def build_fused(cfg, ebs, num_devices=N_CORES, debug_outputs=False):
    npc, ngroups = cfg["npc"], cfg["ngroups"]
    npc_pad, nt1, npad1 = cfg["npc_pad"], cfg["nt1"], cfg["npad1"]
    losplit, lo_rows, hi_rows = cfg["losplit"], cfg["lo_rows"], cfg["hi_rows"]
    lo_sent, hi_sent = cfg["lo_sent"], cfg["hi_sent"]
    sent2 = cfg["sent2"]
    n_cores = cfg["n_cores"]
    e1lo, e1hi, e2lo, e2hi = ebs
    ebt1, ebt2 = e1lo + e1hi, e2lo + e2hi
    lo_tiles = losplit // P
    halfc = n_cores // 2

    nc = bacc.Bacc("TRN2", target_bir_lowering=False, debug=False,
                   num_devices=num_devices)
    xT_ap = nc.dram_tensor("xT16", [2, P, npad1], F16, kind="ExternalInput").ap()
    w1_ap = nc.dram_tensor("w1ext", [D, RW1], F16, kind="ExternalInput").ap()
    b1_ap = nc.dram_tensor("b1", [HC], F32, kind="ExternalInput").ap()
    w2_ap = nc.dram_tensor("w2ext", [HC, RW2], F16, kind="ExternalInput").ap()
    b2_ap = nc.dram_tensor("b2", [OUT], F32, kind="ExternalInput").ap()
    lo1_ap = nc.dram_tensor("lo1", [P, ngroups * e1lo * 8], I16,
                            kind="ExternalInput").ap()
    hi1_ap = nc.dram_tensor("hi1", [P, ngroups * e1hi * 8], I16,
                            kind="ExternalInput").ap()
    dstl1_ap = nc.dram_tensor("dstl1", [P, ngroups * ebt1], F16,
                              kind="ExternalInput").ap()
    lo2_ap = nc.dram_tensor("lo2", [P, ngroups * e2lo * 8], I16,
                            kind="ExternalInput").ap()
    hi2_ap = nc.dram_tensor("hi2", [P, ngroups * e2hi * 8], I16,
                            kind="ExternalInput").ap()
    dstl2_ap = nc.dram_tensor("dstl2", [P, ngroups * ebt2], F16,
                              kind="ExternalInput").ap()
    ni_ap = nc.dram_tensor("nodeidx", [P, ngroups], mybir.dt.int32,
                           kind="ExternalInput").ap()
    de1_ap = nc.dram_tensor("dste1", [P, ngroups * ebt1 * 8], I16,
                            kind="ExternalInput").ap()
    de2_ap = nc.dram_tensor("dste2", [P, ngroups * ebt2 * 8], I16,
                            kind="ExternalInput").ap()
    y_ap = nc.dram_tensor("y", [npc_pad, OUT], F32, kind="ExternalOutput").ap()

    hx1_lo = nc.dram_tensor("hx1_lo", [lo_rows, TRW1], F16).ap()
    hx1_hi = nc.dram_tensor("hx1_hi", [hi_rows, TRW1], F16).ap()
    ad1 = nc.dram_tensor("ad1", [npad1, H], F16).ap()
    ad1_loc = nc.dram_tensor("ad1_loc", [npc_pad, TRW2], F16).ap()
    ad2_loc = nc.dram_tensor("ad2_loc", [npc_pad, TRW2], F16).ap()
    gh = ngroups // 2
    cc_in = nc.dram_tensor("cc_in", [npc_pad, RW2], F16).ap()
    hx2f_a = nc.dram_tensor("hx2f_a", [n_cores, gh * P, RW2], F16,
                            addr_space="Shared").ap()
    hx2f_b = nc.dram_tensor("hx2f_b", [n_cores, (ngroups - gh) * P, RW2],
                            F16, addr_space="Shared").ap()
    hx2p_lo = nc.dram_tensor("hx2p_lo", [halfc * npc_pad, TRW2], F16).ap()
    hx2p_hi = nc.dram_tensor("hx2p_hi", [halfc * npc_pad, TRW2], F16).ap()
    if debug_outputs:
        dbg_cc = nc.dram_tensor("dbg_cc", [npc_pad, RW2], F16,
                                kind="ExternalOutput").ap()

    TB = 8

    with tile.TileContext(nc) as tc:
        with tc.tile_pool(name="const", bufs=1) as cpool:
            ident16 = cpool.tile([P, P], F16)
            make_identity(nc, ident16[:])
            iota_f = cpool.tile([P, P], F16)
            nc.gpsimd.iota(iota_f[:], pattern=[[1, P]], base=0,
                           channel_multiplier=0,
                           allow_small_or_imprecise_dtypes=True)
            w1_sb = cpool.tile([P, 2, RW1], F16)
            nc.sync.dma_start(out=w1_sb[:, 0, :], in_=w1_ap[0:P, :])
            nc.sync.dma_start(out=w1_sb[:, 1, :], in_=w1_ap[P:2 * P, :])
            w2_sb = cpool.tile([P, 2, RW2], F16)
            nc.sync.dma_start(out=w2_sb[:, 0, :], in_=w2_ap[0:P, :])
            nc.sync.dma_start(out=w2_sb[:, 1, :], in_=w2_ap[P:2 * P, :])
            b1bc = cpool.tile([P, HC], F32)
            for hh in range(H):
                nc.sync.dma_start(
                    out=b1bc[:].rearrange("p (c h) -> p c h", h=H)[:, :, hh],
                    in_=b1_ap[hh * CH:(hh + 1) * CH][None, :]
                    .to_broadcast([P, CH]))
            b2bc = cpool.tile([P, OUT], F32)
            nc.sync.dma_start(out=b2bc[:],
                              in_=b2_ap[None, :].to_broadcast([P, OUT]))
            lo1_sb = cpool.tile([P, ngroups * e1lo * 8], I16)
            nc.sync.dma_start(out=lo1_sb[:], in_=lo1_ap[:])
            hi1_sb = cpool.tile([P, ngroups * e1hi * 8], I16)
            nc.sync.dma_start(out=hi1_sb[:], in_=hi1_ap[:])
            dstl1 = cpool.tile([P, ngroups * ebt1], F16)
            nc.sync.dma_start(out=dstl1[:], in_=dstl1_ap[:])
            lo2_sb = cpool.tile([P, ngroups * e2lo * 8], I16)
            nc.sync.dma_start(out=lo2_sb[:], in_=lo2_ap[:])
            hi2_sb = cpool.tile([P, ngroups * e2hi * 8], I16)
            nc.sync.dma_start(out=hi2_sb[:], in_=hi2_ap[:])
            dstl2 = cpool.tile([P, ngroups * ebt2], F16)
            nc.sync.dma_start(out=dstl2[:], in_=dstl2_ap[:])
            nodei = cpool.tile([P, ngroups], mybir.dt.int32)
            nc.sync.dma_start(out=nodei[:], in_=ni_ap[:])
            de1_sb = cpool.tile([P, ngroups * ebt1 * 8], I16)
            nc.sync.dma_start(out=de1_sb[:], in_=de1_ap[:])
            de2_sb = cpool.tile([P, ngroups * ebt2 * 8], I16)
            nc.sync.dma_start(out=de2_sb[:], in_=de2_ap[:])
            hx2_sb = cpool.tile([P, ngroups, RW2], F16)
            nshift = cpool.tile([P, 1], F32)
            nc.gpsimd.memset(nshift[:], -SHIFT1)
            sent_row = cpool.tile([P, RW1], F16)
            nc.vector.memset(sent_row[:], 0.0)
            nc.vector.memset(sent_row[:, HC:HC + H], SENT_AS)

            # ---------------- phase A1: hx1 tables = x @ W1ext -------------
            with (
                tc.tile_pool(name="pa_sbuf", bufs=3) as spool,
                tc.tile_pool(name="pa_out", bufs=3) as opool,
                tc.tile_pool(name="pa_psum", bufs=4, space="PSUM") as pps,
            ):
                for t0 in range(0, nt1, TB):
                    tb = min(TB, nt1 - t0)
                    xt = spool.tile([P, 2, TB * P], F16, tag="xt")
                    for kk in range(2):
                        nc.sync.dma_start(
                            out=xt[:, kk, :tb * P],
                            in_=xT_ap[kk, :, t0 * P:(t0 + tb) * P])
                    stage = opool.tile([P, TB, RW1], F16, tag="stage")
                    for ti in range(tb):
                        ps = pps.tile([P, RW1], F32, tag="ps")
                        for kk in range(2):
                            nc.tensor.matmul(
                                ps[:], lhsT=xt[:, kk, ti * P:(ti + 1) * P],
                                rhs=w1_sb[:, kk, :],
                                start=(kk == 0), stop=(kk == 1))
                        if ti % 2 == 0:
                            nc.scalar.copy(stage[:, ti, :], ps[:])
                        else:
                            nc.vector.tensor_copy(stage[:, ti, :], ps[:])
                    # route tiles to the lo/hi tables
                    spans = []
                    if t0 < lo_tiles:
                        n_lo = min(tb, lo_tiles - t0)
                        spans.append((hx1_lo, t0 * P, 0, n_lo))
                        if n_lo < tb:
                            spans.append((hx1_hi, 0, n_lo, tb - n_lo))
                    else:
                        spans.append((hx1_hi, t0 * P - losplit, 0, tb))
                    for tab, r0, ti0, ntl in spans:
                        nc.sync.dma_start(
                            out=tab[r0:r0 + ntl * P, :RW1].rearrange(
                                "(t p) w -> p t w", p=P),
                            in_=stage[:, ti0:ti0 + ntl, :])
                    nc.sync.dma_start(
                        out=ad1[t0 * P:(t0 + tb) * P, :].rearrange(
                            "(t p) w -> p t w", p=P),
                        in_=stage[:, :tb, PAY1:PAY1 + H])
                # sentinel rows: full zero row with as=-30000 in lo; the hi
                # sentinel (a real pad row, h already 0) gets just as cols
                nc.sync.dma_start(out=hx1_lo[lo_sent:lo_sent + 1, :RW1],
                                  in_=sent_row[0:1, :])
                nc.sync.dma_start(
                    out=hx1_hi[hi_sent:hi_sent + 1, HC:HC + H],
                    in_=sent_row[0:1, HC:HC + H])

            # ---------------- phase B1: layer-1 aggregation + W2 fold ------
            with (
                tc.tile_pool(name="pb_gather", bufs=3) as gpool,
                tc.tile_pool(name="pb_work", bufs=3) as wpool,
                tc.tile_pool(name="pb_ep", bufs=2) as epool,
                tc.tile_pool(name="pb_psum", bufs=2, space="PSUM") as upps,
                tc.tile_pool(name="pb_psumT", bufs=2, space="PSUM") as tpps,
            ):
                for g in range(ngroups):
                    adg_w = epool.tile([P, TRW2], F16, tag="adg_w")
                    nc.gpsimd.indirect_dma_start(
                        out=adg_w[:, :H], out_offset=None, in_=ad1,
                        in_offset=bass.IndirectOffsetOnAxis(
                            ap=nodei[:, g:g + 1], axis=0))
                    nc.sync.dma_start(
                        out=ad1_loc[g * P:(g + 1) * P, :H], in_=adg_w[:, :H])
                for g in range(ngroups):
                    pay = gpool.tile([P, ebt1, TRW1], F16, tag="pay")
                    nc.gpsimd.dma_gather(
                        out_ap=pay[:, :e1lo, :],
                        in_ap=hx1_lo[:],
                        idxs_ap=lo1_sb[:, g * e1lo * 8:(g + 1) * e1lo * 8],
                        num_idxs=e1lo * P, num_idxs_reg=e1lo * P,
                        elem_size=TRW1, single_packet=False)
                    nc.gpsimd.dma_gather(
                        out_ap=pay[:, e1lo:, :],
                        in_ap=hx1_hi[:],
                        idxs_ap=hi1_sb[:, g * e1hi * 8:(g + 1) * e1hi * 8],
                        num_idxs=e1hi * P, num_idxs_reg=e1hi * P,
                        elem_size=TRW1, single_packet=False)
                    ade = gpool.tile([P, ebt1, TRW2], F16, tag="ade")
                    nc.gpsimd.dma_gather(
                        out_ap=ade[:],
                        in_ap=ad1_loc[:],
                        idxs_ap=de1_sb[:, g * ebt1 * 8:(g + 1) * ebt1 * 8],
                        num_idxs=ebt1 * P, num_idxs_reg=ebt1 * P,
                        elem_size=TRW2, single_packet=False)
                    cs1 = slice(g * ebt1, (g + 1) * ebt1)
                    msb = wpool.tile([P, ebt1, P], F16, tag="msb")
                    nc.vector.tensor_tensor(
                        out=msb[:],
                        in0=iota_f[:, None, :].to_broadcast([P, ebt1, P]),
                        in1=dstl1[:, cs1][:, :, None].to_broadcast(
                            [P, ebt1, P]),
                        op=mybir.AluOpType.is_equal)
                    z = wpool.tile([P, ebt1, H], F32, tag="z")
                    nc.vector.tensor_tensor(
                        out=z[:], in0=pay[:, :, HC:HC + H],
                        in1=ade[:, :, :H],
                        op=mybir.AluOpType.add)
                    z2 = wpool.tile([P, ebt1, H], F32, tag="z2")
                    nc.vector.tensor_scalar_mul(z2[:], z[:], 0.2)
                    lr = wpool.tile([P, ebt1, H], F32, tag="lr")
                    nc.vector.tensor_tensor(out=lr[:], in0=z[:], in1=z2[:],
                                            op=mybir.AluOpType.max)
                    gsb = wpool.tile([P, ebt1, PAY1], F16, tag="gsb")
                    nc.scalar.activation(
                        out=gsb[:, :, HC:], in_=lr[:],
                        func=mybir.ActivationFunctionType.Exp,
                        bias=nshift[:])
                    nc.vector.tensor_tensor(
                        out=gsb[:, :, :HC].rearrange(
                            "p j (c h) -> p j c h", h=H),
                        in0=pay[:, :, :HC].rearrange(
                            "p j (c h) -> p j c h", h=H),
                        in1=gsb[:, :, None, HC:].to_broadcast([P, ebt1, CH, H]),
                        op=mybir.AluOpType.mult)
                    u_ps = upps.tile([P, PAY1], F32, tag="u_ps")
                    for j in range(ebt1):
                        nc.tensor.matmul(u_ps[:], lhsT=msb[:, j, :],
                                         rhs=gsb[:, j, :],
                                         start=(j == 0), stop=(j == ebt1 - 1))
                    # epilogue
                    s_sb = epool.tile([P, H], F32, tag="s_sb")
                    nc.vector.tensor_scalar_add(s_sb[:], u_ps[:, HC:], 1e-16)
                    r_sb = epool.tile([P, H], F32, tag="r_sb")
                    nc.vector.reciprocal(r_sb[:], s_sb[:])
                    zt = epool.tile([P, HC], F32, tag="zt")
                    nc.vector.tensor_tensor(
                        out=zt[:].rearrange("p (c h) -> p c h", h=H),
                        in0=u_ps[:, :HC].rearrange("p (c h) -> p c h", h=H),
                        in1=r_sb[:][:, None, :].to_broadcast([P, CH, H]),
                        op=mybir.AluOpType.mult)
                    zb = epool.tile([P, HC], F16, tag="zb")
                    nc.vector.tensor_tensor(out=zb[:], in0=zt[:], in1=b1bc[:],
                                            op=mybir.AluOpType.add)
                    t1 = epool.tile([P, HC], F16, tag="t1")
                    nc.vector.tensor_scalar(out=t1[:], in0=zb[:], scalar1=0.0,
                                            scalar2=None,
                                            op0=mybir.AluOpType.min)
                    t2 = epool.tile([P, HC], F16, tag="t2")
                    nc.scalar.activation(out=t2[:], in_=t1[:],
                                         func=mybir.ActivationFunctionType.Exp)
                    t3 = epool.tile([P, HC], F16, tag="t3")
                    nc.vector.tensor_scalar_add(t3[:], t2[:], -1.0)
                    h16 = epool.tile([P, HC], F16, tag="h16")
                    nc.vector.tensor_tensor(out=h16[:], in0=zb[:], in1=t3[:],
                                            op=mybir.AluOpType.max)
                    h2_ps = tpps.tile([P, RW2], F32, tag="h2_ps")
                    for kk in range(2):
                        hT_ps = tpps.tile([P, P], F16, tag="hT_ps")
                        nc.tensor.transpose(hT_ps[:],
                                            h16[:, kk * P:(kk + 1) * P],
                                            ident16[:])
                        hT_sb = epool.tile([P, P], F16, tag="hT_sb")
                        nc.vector.tensor_copy(hT_sb[:], hT_ps[:])
                        nc.tensor.matmul(h2_ps[:], lhsT=hT_sb[:],
                                         rhs=w2_sb[:, kk, :],
                                         start=(kk == 0), stop=(kk == 1))
                    nc.scalar.copy(hx2_sb[:, g, :], h2_ps[:])
                    if g == gh - 1:
                        # first-half allgather overlaps the remaining groups
                        nc.sync.dma_start(
                            out=cc_in[:gh * P].rearrange(
                                "(g p) w -> p g w", p=P),
                            in_=hx2_sb[:, :gh, :])
                        nc.gpsimd.collective_compute(
                            "AllGather", mybir.AluOpType.bypass,
                            replica_groups=[list(range(n_cores))],
                            ins=[cc_in[:gh * P]], outs=[hx2f_a[:]])
                nc.sync.dma_start(
                    out=cc_in[gh * P:].rearrange("(g p) w -> p g w", p=P),
                    in_=hx2_sb[:, gh:, :])
                # layer-2 sentinel: as2 = -30000 on the first pad row
                assert sent2 >= gh * P
                nc.sync.dma_start(
                    out=cc_in[sent2:sent2 + 1, PAY2 - 1:PAY2],
                    in_=sent_row[0:1, HC:HC + 1])
                nc.gpsimd.collective_compute(
                    "AllGather", mybir.AluOpType.bypass,
                    replica_groups=[list(range(n_cores))],
                    ins=[cc_in[gh * P:]], outs=[hx2f_b[:]])
                nc.sync.dma_start(
                    out=ad2_loc[:, :1].rearrange("(g p) w -> p g w", p=P),
                    in_=hx2_sb[:, :, PAY2:PAY2 + 1])

            # ------------- expand hx2f into padded lo/hi tables ------------
            with tc.tile_pool(name="px", bufs=4) as xpool:
                for r in range(n_cores):
                    xt2 = xpool.tile([P, gh, RW2], F16, tag="xt2")
                    nc.sync.dma_start(
                        out=xt2[:],
                        in_=hx2f_a[r].rearrange("(g p) w -> p g w", p=P))
                    tab = hx2p_lo if r < halfc else hx2p_hi
                    r0 = (r % halfc) * npc_pad
                    nc.sync.dma_start(
                        out=tab[r0:r0 + gh * P, :PAY2 + 1].rearrange(
                            "(g p) w -> p g w", p=P),
                        in_=xt2[:, :, :PAY2 + 1])
                for r in range(n_cores):
                    xt3 = xpool.tile([P, ngroups - gh, RW2], F16, tag="xt3")
                    nc.sync.dma_start(
                        out=xt3[:],
                        in_=hx2f_b[r].rearrange("(g p) w -> p g w", p=P))
                    tab = hx2p_lo if r < halfc else hx2p_hi
                    r0 = (r % halfc) * npc_pad
                    nc.sync.dma_start(
                        out=tab[r0 + gh * P:r0 + npc_pad, :PAY2 + 1].rearrange(
                            "(g p) w -> p g w", p=P),
                        in_=xt3[:, :, :PAY2 + 1])
                if debug_outputs:
                    dt2 = xpool.tile([P, ngroups, RW2], F16, tag="dt2")
                    nc.sync.dma_start(
                        out=dt2[:],
                        in_=cc_in[:].rearrange("(g p) w -> p g w", p=P))
                    nc.sync.dma_start(
                        out=dbg_cc[:].rearrange("(g p) w -> p g w", p=P),
                        in_=dt2[:])

            # ---------------- phase B2: layer-2 aggregation ----------------
            with (
                tc.tile_pool(name="p2_gather", bufs=3) as g2pool,
                tc.tile_pool(name="p2_work", bufs=3) as w2pool,
                tc.tile_pool(name="p2_ep", bufs=2) as e2pool,
                tc.tile_pool(name="p2_psum", bufs=2, space="PSUM") as u2ps,
            ):
                for g in range(ngroups):
                    pay = g2pool.tile([P, ebt2, TRW2], F16, tag="pay2")
                    nc.gpsimd.dma_gather(
                        out_ap=pay[:, :e2lo, :],
                        in_ap=hx2p_lo[:],
                        idxs_ap=lo2_sb[:, g * e2lo * 8:(g + 1) * e2lo * 8],
                        num_idxs=e2lo * P, num_idxs_reg=e2lo * P,
                        elem_size=TRW2, single_packet=False)
                    nc.gpsimd.dma_gather(
                        out_ap=pay[:, e2lo:, :],
                        in_ap=hx2p_hi[:],
                        idxs_ap=hi2_sb[:, g * e2hi * 8:(g + 1) * e2hi * 8],
                        num_idxs=e2hi * P, num_idxs_reg=e2hi * P,
                        elem_size=TRW2, single_packet=False)
                    ade = g2pool.tile([P, ebt2, TRW2], F16, tag="ade2")
                    nc.gpsimd.dma_gather(
                        out_ap=ade[:],
                        in_ap=ad2_loc[:],
                        idxs_ap=de2_sb[:, g * ebt2 * 8:(g + 1) * ebt2 * 8],
                        num_idxs=ebt2 * P, num_idxs_reg=ebt2 * P,
                        elem_size=TRW2, single_packet=False)
                    cs2 = slice(g * ebt2, (g + 1) * ebt2)
                    msb = w2pool.tile([P, ebt2, P], F16, tag="msb2")
                    nc.vector.tensor_tensor(
                        out=msb[:],
                        in0=iota_f[:, None, :].to_broadcast([P, ebt2, P]),
                        in1=dstl2[:, cs2][:, :, None].to_broadcast(
                            [P, ebt2, P]),
                        op=mybir.AluOpType.is_equal)
                    z = w2pool.tile([P, ebt2, 1], F32, tag="z")
                    nc.vector.tensor_tensor(out=z[:],
                                            in0=pay[:, :, OUT:OUT + 1],
                                            in1=ade[:, :, :1],
                                            op=mybir.AluOpType.add)
                    z2 = w2pool.tile([P, ebt2, 1], F32, tag="z2")
                    nc.vector.tensor_scalar_mul(z2[:], z[:], 0.2)
                    lr = w2pool.tile([P, ebt2, 1], F32, tag="lr")
                    nc.vector.tensor_tensor(out=lr[:], in0=z[:], in1=z2[:],
                                            op=mybir.AluOpType.max)
                    gsb = w2pool.tile([P, ebt2, PAY2], F16, tag="gsb2")
                    nc.scalar.activation(
                        out=gsb[:, :, OUT:], in_=lr[:],
                        func=mybir.ActivationFunctionType.Exp, bias=0.0)
                    nc.vector.tensor_tensor(
                        out=gsb[:, :, :OUT],
                        in0=pay[:, :, :OUT],
                        in1=gsb[:, :, OUT:].to_broadcast([P, ebt2, OUT]),
                        op=mybir.AluOpType.mult)
                    u_ps = u2ps.tile([P, PAY2], F32, tag="u_ps2")
                    for j in range(ebt2):
                        nc.tensor.matmul(u_ps[:], lhsT=msb[:, j, :],
                                         rhs=gsb[:, j, :],
                                         start=(j == 0), stop=(j == ebt2 - 1))
                    s_sb = e2pool.tile([P, 1], F32, tag="s_sb2")
                    nc.vector.tensor_scalar_add(s_sb[:], u_ps[:, OUT:], 1e-16)
                    r_sb = e2pool.tile([P, 1], F32, tag="r_sb2")
                    nc.vector.reciprocal(r_sb[:], s_sb[:])
                    y_sb = e2pool.tile([P, OUT], F32, tag="y_sb")
                    nc.scalar.activation(
                        out=y_sb[:], in_=u_ps[:, :OUT],
                        func=mybir.ActivationFunctionType.Copy,
                        scale=r_sb[:, 0:1])
                    yb = e2pool.tile([P, OUT], F32, tag="yb")
                    nc.vector.tensor_tensor(out=yb[:], in0=y_sb[:],
                                            in1=b2bc[:],
                                            op=mybir.AluOpType.add)
                    nc.sync.dma_start(out=y_ap[g * P:(g + 1) * P, :],
                                      in_=yb[:])
    nc.compile()
    return nc


# --------------------------------------------------------------------------
# host-side input prep
# --------------------------------------------------------------------------

def prep_inputs(inputs, cfg, tables):
    x = np.asarray(inputs["x"], np.float32)
    npad1 = cfg["npad1"]
    xT = np.zeros((D, npad1), np.float16)
    xT[:, :cfg["n_valid"]] = x.T.astype(np.float16)
    xT16 = np.ascontiguousarray(xT.reshape(2, P, npad1))
    W1ext, W2ext = fold_weights(
        inputs["W1"], inputs["a_src1"], inputs["a_dst1"],
        inputs["W2"], inputs["a_src2"], inputs["a_dst2"])
    b1 = np.asarray(inputs["b1"], np.float32)
    b2 = np.asarray(inputs["b2"], np.float32)
    in_maps = [dict(
        xT16=xT16, w1ext=W1ext, b1=b1, w2ext=W2ext, b2=b2,
        lo1=t["lo1"], hi1=t["hi1"], dstl1=t["dstl1"], dste1=t["dste1"],
        lo2=t["lo2"], hi2=t["hi2"], dstl2=t["dstl2"], dste2=t["dste2"],
        nodeidx=t["nodeidx"],
    ) for t in tables]
    return in_maps


_CACHE = {}


def get_nc(cfg, ebs):
    key = (cfg["n_valid"], cfg["n_cores"], ebs)
    if key not in _CACHE:
        _CACHE[key] = build_fused(cfg, ebs)
    return _CACHE[key]


def _run_with_retry(nc, in_maps, tries=3):
    from concourse.bass_utils import run_bass_kernel_spmd
    last = None
    for attempt in range(tries):
        try:
            return run_bass_kernel_spmd(nc, in_maps,
                                        core_ids=list(range(len(in_maps))))
        except Exception as e:  # noqa: BLE001 - retry any runtime failure
            last = e
            import time as _time
            _time.sleep(2.0 * (attempt + 1))
    raise last


def kernel(**inputs):
    """Full-input GAT kernel on 8 Trainium2 NeuronCores.

    Takes the unsharded inputs of reference.setup_inputs(), distributes the
    work across 8 cores (dst-node graph partition) in a single fused launch
    with an on-device AllGather between the layers, and returns the full
    [50000, 32] float32 output.
    """
    x = np.asarray(inputs["x"], np.float32)
    ei = np.asarray(inputs["edge_index"])
    N = x.shape[0]
    cfg = make_cfg(N)
    src = np.concatenate([ei[0].astype(np.int64),
                          np.arange(N, dtype=np.int64)])
    dst = np.concatenate([ei[1].astype(np.int64),
                          np.arange(N, dtype=np.int64)])
    tables, ebs = build_edge_tables(src, dst, cfg)
    nc = get_nc(cfg, ebs)
    in_maps = prep_inputs(inputs, cfg, tables)
    res = _run_with_retry(nc, in_maps)
    npc = cfg["npc"]
    y = np.concatenate([res.results[k]["y"][:npc]
                        for k in range(cfg["n_cores"])], axis=0)
    return y.astype(np.float32)


# revision 16
# speedup vs baseline: 1.0393x; 1.0393x over previous
"""Two-layer GAT on 8 Trainium2 cores via Bass/Tile — fused single launch.

Strategy (dst-node graph partition, per the sharding hint):
- Nodes are split into 8 contiguous ranges (6250 per core); every edge is
  owned by the core that owns its dst node.
- Single launch per core:
  * Phase A1 (redundant on every core): hx1 = x @ [W1p | W1@Asrc] written as
    fp16 tables with 768B rows [h (c,h-interleaved) 256 | a_src.h 8 | pad],
    split lo/hi at row 32512 so dma_gather's int16 indices stay in range;
    a_dst.h goes to a compact ad1 [npad1, 8] side table. Sentinel rows have
    a_src = -30000 (dummy edges gather them; exp == 0).
  * Phase B1 (layer-1 edge aggregation, per 128-dst-node group):
    two dma_gathers (lo/hi) fetch all edge payload rows [128, EBT, 384].
    The group's a_dst block adg [128, 8] comes from one plain strided DMA
    (dst nodes of a group are contiguous); per-edge a_dst is M_T @ adg on
    the PE, where M_T (and M for the scatter) are built by two batched
    is_equal ops from the dst-offset table. Batched DVE/Act ops compute
    w = exp(lrelu(as+ad) - 6) and G = [h*w | w]; EBT PE matmuls accumulate
    U += M_j.T @ G_j in PSUM. Epilogue: h1 = elu(U/(sum)+b1), transposed on
    PE and folded through W2 immediately: hx2 row = [h1@W2 | a2s.h1 | a2d.h1]
    into an SBUF shard table (the layer-2 dense phase is fused in here).
  * AllGather (on-device collective) of the per-core hx2 shard [6272, 36]
    -> [8, 6272, 36] Shared DRAM, then expanded on-device into two padded
    256B-row tables (ranks 0-3 / 4-7) for int16 dma_gather indexing.
  * Phase B2: per-group aggregation as B1 (adg2 is already in SBUF from the
    fold); writes y rows [6272, 32] f32.
- Host: slices the 8 y shards to 6250 rows each and concatenates.
"""

import sys
for _p in ("/opt/trn_rl_repo",):
    if _p not in sys.path:
        sys.path.append(_p)

import math
import numpy as np

import concourse.bass as bass
import concourse.mybir as mybir
import concourse.tile as tile
from concourse import bacc
from concourse.masks import make_identity

F32 = mybir.dt.float32
F16 = mybir.dt.float16
I16 = mybir.dt.int16

N_CORES = 8
D = 256
HC = 256
H = 8
CH = 32
OUT = 32
P = 128
TRW1 = 384       # hx1 table row width (fp16) = 768B (dma_gather granularity)
RW1 = 272        # written row prefix: 256 h + 8 as + 8 ad
PAY1 = 264
TRW2 = 128       # hx2 padded table row width (fp16) = 256B
RW2 = 36         # compact hx2 row: 32 h2 + 1 as2 + 1 ad2 + 2 pad
PAY2 = 33
SHIFT1 = 6.0
SENT_AS = -30000.0
LOSPLIT1 = 32512  # 254 * 128


def make_cfg(n_valid, n_cores=N_CORES):
    npc = n_valid // n_cores
    assert npc * n_cores == n_valid
    ngroups = math.ceil(npc / P)
    npc_pad = ngroups * P
    assert npc_pad > npc, "need a pad row for the layer-2 sentinel"
    nt1 = math.ceil(n_valid / P)
    npad1 = nt1 * P
    losplit = min(LOSPLIT1, (nt1 // 2) * P)
    lo_rows = losplit + P             # + sentinel row block
    hi_rows = npad1 - losplit
    assert losplit <= 32767 and hi_rows <= 32767
    half = (n_cores // 2) * npc_pad   # L2 lo/hi split (remapped rows)
    assert half <= 32767 and n_cores * npc_pad - half <= 32767
    return dict(
        n_valid=n_valid, n_cores=n_cores, npc=npc, ngroups=ngroups,
        npc_pad=npc_pad, nt1=nt1, npad1=npad1,
        losplit=losplit, lo_rows=lo_rows, hi_rows=hi_rows,
        lo_sent=losplit, hi_sent=n_valid - losplit,
        half2=half, sent2=npc,
    )


def fold_weights(W1, a_src1, a_dst1, W2, a_src2, a_dst2):
    W1 = np.asarray(W1, np.float32)
    a_src1 = np.asarray(a_src1, np.float32)
    a_dst1 = np.asarray(a_dst1, np.float32)
    A_src = np.zeros((HC, H), np.float32)
    A_dst = np.zeros((HC, H), np.float32)
    for h in range(H):
        A_src[h * CH:(h + 1) * CH, h] = a_src1[h]
        A_dst[h * CH:(h + 1) * CH, h] = a_dst1[h]
    # interleave the h-block columns to (c, h) order so DVE broadcast muls
    # on-device have a packed last dim
    perm = (np.arange(CH)[:, None] + np.arange(H)[None, :] * CH).reshape(-1)
    W1ext = np.concatenate([W1[:, perm], W1 @ A_src, W1 @ A_dst], axis=1)
    W2 = np.asarray(W2, np.float32)
    W2e = np.concatenate(
        [W2, W2 @ np.asarray(a_src2, np.float32).T,
         W2 @ np.asarray(a_dst2, np.float32).T,
         np.zeros((HC, RW2 - PAY2 - 1), np.float32)], axis=1)   # [256, 36]
    # rows of W2e must match the (c,h)-permuted h1 produced by the PE
    # transposes: contraction index kk*128 + (c - 16*kk)*8 + h <-> row h*32+c
    rperm = np.empty(HC, np.int64)
    i = 0
    for kk in range(2):
        for cl in range(16):
            for h in range(H):
                rperm[i] = h * CH + kk * 16 + cl
                i += 1
    W2ext = W2e[rperm]
    return W1ext.astype(np.float16), W2ext.astype(np.float16)


def _wrap16(arr):
    # dma_gather index layout: ordinal i -> [i % 16, i // 16], x8 rows
    n = arr.size
    return np.tile(arr.reshape(n // 16, 16).T, (8, 1)).astype(np.int16)


def _bucket_tables(src_rows, dst_local, grp, ngroups, losplit, lo_sent,
                   hi_sent, eb_lo, eb_hi):
    """Bucket one core's edges into per-group lo/hi blocks.

    src_rows: table row index per edge; dst_local: dst offset within group
    [0,128); grp: group id per edge. Returns (lo_idx, hi_idx, dstl) with
    lo_idx/hi_idx int16-wrapped [128, ngroups*eb*8] and dstl f16
    [128, ngroups*(eb_lo+eb_hi)].
    """
    islo_all = src_rows < losplit
    ebt = eb_lo + eb_hi
    lo_a = np.full((ngroups, eb_lo * P), lo_sent, np.int64)
    hi_a = np.full((ngroups, eb_hi * P), hi_sent, np.int64)
    dste_a = np.zeros((ngroups, ebt * P), np.int64)
    dstl = np.zeros((P, ngroups * ebt), np.float16)
    order = np.argsort(grp, kind="stable")
    src_rows, dst_local, grp, islo_all = (src_rows[order], dst_local[order],
                                          grp[order], islo_all[order])
    gstart = np.searchsorted(grp, np.arange(ngroups + 1))
    for g in range(ngroups):
        a, b = gstart[g], gstart[g + 1]
        sl, dl, lo = src_rows[a:b], dst_local[a:b], islo_all[a:b]
        nlo = int(lo.sum())
        nhi = (b - a) - nlo
        assert nlo <= eb_lo * P and nhi <= eb_hi * P
        lo_a[g, :nlo] = sl[lo]
        hi_a[g, :nhi] = sl[~lo] - losplit
        ilo = np.arange(nlo)
        dstl[ilo % P, g * ebt + ilo // P] = dl[lo].astype(np.float16)
        dste_a[g, :nlo] = g * P + dl[lo]
        ihi = np.arange(nhi)
        dstl[ihi % P, g * ebt + eb_lo + ihi // P] = dl[~lo].astype(np.float16)
        dste_a[g, eb_lo * P:eb_lo * P + nhi] = g * P + dl[~lo]
    lo_idx = np.concatenate([_wrap16(lo_a[g]) for g in range(ngroups)], axis=1)
    hi_idx = np.concatenate([_wrap16(hi_a[g]) for g in range(ngroups)], axis=1)
    dste = np.concatenate([_wrap16(dste_a[g]) for g in range(ngroups)], axis=1)
    return lo_idx, hi_idx, dstl, dste


def build_edge_tables(src, dst, cfg, ebs=None):
    """Per-core gather tables for both layers. ebs = (eb1_lo, eb1_hi,
    eb2_lo, eb2_hi) or None to size from the data."""
    npc, ngroups = cfg["npc"], cfg["ngroups"]
    npc_pad = cfg["npc_pad"]
    losplit, lo_sent, hi_sent = cfg["losplit"], cfg["lo_sent"], cfg["hi_sent"]
    half2, sent2 = cfg["half2"], cfg["sent2"]
    n_cores = cfg["n_cores"]

    src = np.asarray(src, np.int64)
    dst = np.asarray(dst, np.int64)
    core = dst // npc

    per_core = []
    for k in range(n_cores):
        m = core == k
        s_k = src[m]
        d_k = dst[m] - k * npc
        g_k = d_k // P
        dl_k = d_k - g_k * P
        s2_k = (s_k // npc) * npc_pad + (s_k % npc)   # remapped L2 rows
        per_core.append((s_k, s2_k, dl_k, g_k))

    if ebs is not None:
        e1lo, e1hi, e2lo, e2hi = ebs
    else:
        e1lo = e1hi = e2lo = e2hi = 1
        for s_k, s2_k, dl_k, g_k in per_core:
            islo1 = s_k < losplit
            c = np.bincount(g_k[islo1], minlength=ngroups).max()
            e1lo = max(e1lo, math.ceil(c / P))
            c = np.bincount(g_k[~islo1], minlength=ngroups).max()
            e1hi = max(e1hi, math.ceil(c / P))
            islo2 = s2_k < half2
            c = np.bincount(g_k[islo2], minlength=ngroups).max()
            e2lo = max(e2lo, math.ceil(c / P))
            c = np.bincount(g_k[~islo2], minlength=ngroups).max()
            e2hi = max(e2hi, math.ceil(c / P))

    tables = []
    for s_k, s2_k, dl_k, g_k in per_core:
        lo1, hi1, dstl1, dste1 = _bucket_tables(
            s_k, dl_k, g_k, ngroups, losplit, lo_sent, hi_sent, e1lo, e1hi)
        # L2 sentinel: every rank's local pad row `sent2` has as2=-30000;
        # lo dummies -> rank0's (row sent2), hi dummies -> rank halfc's
        # (local row sent2 within the hi table)
        lo2, hi2, dstl2, dste2 = _bucket_tables(
            s2_k, dl_k, g_k, ngroups, half2, sent2, sent2, e2lo, e2hi)
        tables.append(dict(lo1=lo1, hi1=hi1, dstl1=dstl1, dste1=dste1,
                           lo2=lo2, hi2=hi2, dstl2=dstl2, dste2=dste2))
    npad1 = cfg["npad1"]
    for k, t in enumerate(tables):
        ni = (k * npc + np.arange(P)[:, None]
              + P * np.arange(ngroups)[None, :])
        t["nodeidx"] = np.minimum(ni, npad1 - 1).astype(np.int32)
    return tables, (e1lo, e1hi, e2lo, e2hi)


# --------------------------------------------------------------------------
# fused launch
# --------------------------------------------------------------------------

def build_fused(cfg, ebs, num_devices=N_CORES, debug_outputs=False,
                split_cc=True):
    npc, ngroups = cfg["npc"], cfg["ngroups"]
    npc_pad, nt1, npad1 = cfg["npc_pad"], cfg["nt1"], cfg["npad1"]
    losplit, lo_rows, hi_rows = cfg["losplit"], cfg["lo_rows"], cfg["hi_rows"]
    lo_sent, hi_sent = cfg["lo_sent"], cfg["hi_sent"]
    sent2 = cfg["sent2"]
    n_cores = cfg["n_cores"]
    e1lo, e1hi, e2lo, e2hi = ebs
    ebt1, ebt2 = e1lo + e1hi, e2lo + e2hi
    lo_tiles = losplit // P
    halfc = n_cores // 2

    nc = bacc.Bacc("TRN2", target_bir_lowering=False, debug=False,
                   num_devices=num_devices)
    xT_ap = nc.dram_tensor("xT16", [2, P, npad1], F16, kind="ExternalInput").ap()
    w1_ap = nc.dram_tensor("w1ext", [D, RW1], F16, kind="ExternalInput").ap()
    b1_ap = nc.dram_tensor("b1", [HC], F32, kind="ExternalInput").ap()
    w2_ap = nc.dram_tensor("w2ext", [HC, RW2], F16, kind="ExternalInput").ap()
    b2_ap = nc.dram_tensor("b2", [OUT], F32, kind="ExternalInput").ap()
    lo1_ap = nc.dram_tensor("lo1", [P, ngroups * e1lo * 8], I16,
                            kind="ExternalInput").ap()
    hi1_ap = nc.dram_tensor("hi1", [P, ngroups * e1hi * 8], I16,
                            kind="ExternalInput").ap()
    dstl1_ap = nc.dram_tensor("dstl1", [P, ngroups * ebt1], F16,
                              kind="ExternalInput").ap()
    lo2_ap = nc.dram_tensor("lo2", [P, ngroups * e2lo * 8], I16,
                            kind="ExternalInput").ap()
    hi2_ap = nc.dram_tensor("hi2", [P, ngroups * e2hi * 8], I16,
                            kind="ExternalInput").ap()
    dstl2_ap = nc.dram_tensor("dstl2", [P, ngroups * ebt2], F16,
                              kind="ExternalInput").ap()
    ni_ap = nc.dram_tensor("nodeidx", [P, ngroups], mybir.dt.int32,
                           kind="ExternalInput").ap()
    de1_ap = nc.dram_tensor("dste1", [P, ngroups * ebt1 * 8], I16,
                            kind="ExternalInput").ap()
    de2_ap = nc.dram_tensor("dste2", [P, ngroups * ebt2 * 8], I16,
                            kind="ExternalInput").ap()
    y_ap = nc.dram_tensor("y", [npc_pad, OUT], F32, kind="ExternalOutput").ap()

    hx1_lo = nc.dram_tensor("hx1_lo", [lo_rows, TRW1], F16).ap()
    hx1_hi = nc.dram_tensor("hx1_hi", [hi_rows, TRW1], F16).ap()
    ad1 = nc.dram_tensor("ad1", [npad1, H], F16).ap()
    ad1_loc = nc.dram_tensor("ad1_loc", [npc_pad, TRW2], F16).ap()
    ad2_loc = nc.dram_tensor("ad2_loc", [npc_pad, TRW2], F16).ap()
    gh = ngroups // 2
    cc_in = nc.dram_tensor("cc_in", [npc_pad, RW2], F16).ap()
    hx2f_a = nc.dram_tensor("hx2f_a", [n_cores, gh * P, RW2], F16,
                            addr_space="Shared").ap()
    hx2f_b = nc.dram_tensor("hx2f_b", [n_cores, (ngroups - gh) * P, RW2],
                            F16, addr_space="Shared").ap()
    hx2p_lo = nc.dram_tensor("hx2p_lo", [halfc * npc_pad, TRW2], F16).ap()
    hx2p_hi = nc.dram_tensor("hx2p_hi", [halfc * npc_pad, TRW2], F16).ap()
    if debug_outputs:
        dbg_cc = nc.dram_tensor("dbg_cc", [npc_pad, RW2], F16,
                                kind="ExternalOutput").ap()

    TB = 8

    with tile.TileContext(nc) as tc:
        with tc.tile_pool(name="const", bufs=1) as cpool:
            ident16 = cpool.tile([P, P], F16)
            make_identity(nc, ident16[:])
            iota_f = cpool.tile([P, P], F16)
            nc.gpsimd.iota(iota_f[:], pattern=[[1, P]], base=0,
                           channel_multiplier=0,
                           allow_small_or_imprecise_dtypes=True)
            w1_sb = cpool.tile([P, 2, RW1], F16)
            nc.sync.dma_start(out=w1_sb[:, 0, :], in_=w1_ap[0:P, :])
            nc.sync.dma_start(out=w1_sb[:, 1, :], in_=w1_ap[P:2 * P, :])
            w2_sb = cpool.tile([P, 2, RW2], F16)
            nc.sync.dma_start(out=w2_sb[:, 0, :], in_=w2_ap[0:P, :])
            nc.sync.dma_start(out=w2_sb[:, 1, :], in_=w2_ap[P:2 * P, :])
            b1bc = cpool.tile([P, HC], F32)
            for hh in range(H):
                nc.sync.dma_start(
                    out=b1bc[:].rearrange("p (c h) -> p c h", h=H)[:, :, hh],
                    in_=b1_ap[hh * CH:(hh + 1) * CH][None, :]
                    .to_broadcast([P, CH]))
            b2bc = cpool.tile([P, OUT], F32)
            nc.sync.dma_start(out=b2bc[:],
                              in_=b2_ap[None, :].to_broadcast([P, OUT]))
            lo1_sb = cpool.tile([P, ngroups * e1lo * 8], I16)
            nc.sync.dma_start(out=lo1_sb[:], in_=lo1_ap[:])
            hi1_sb = cpool.tile([P, ngroups * e1hi * 8], I16)
            nc.sync.dma_start(out=hi1_sb[:], in_=hi1_ap[:])
            dstl1 = cpool.tile([P, ngroups * ebt1], F16)
            nc.sync.dma_start(out=dstl1[:], in_=dstl1_ap[:])
            lo2_sb = cpool.tile([P, ngroups * e2lo * 8], I16)
            nc.sync.dma_start(out=lo2_sb[:], in_=lo2_ap[:])
            hi2_sb = cpool.tile([P, ngroups * e2hi * 8], I16)
            nc.sync.dma_start(out=hi2_sb[:], in_=hi2_ap[:])
            dstl2 = cpool.tile([P, ngroups * ebt2], F16)
            nc.sync.dma_start(out=dstl2[:], in_=dstl2_ap[:])
            nodei = cpool.tile([P, ngroups], mybir.dt.int32)
            nc.sync.dma_start(out=nodei[:], in_=ni_ap[:])
            de1_sb = cpool.tile([P, ngroups * ebt1 * 8], I16)
            nc.sync.dma_start(out=de1_sb[:], in_=de1_ap[:])
            de2_sb = cpool.tile([P, ngroups * ebt2 * 8], I16)
            nc.sync.dma_start(out=de2_sb[:], in_=de2_ap[:])
            hx2_sb = cpool.tile([P, ngroups, RW2], F16)
            nshift = cpool.tile([P, 1], F32)
            nc.gpsimd.memset(nshift[:], -SHIFT1)
            sent_row = cpool.tile([P, RW1], F16)
            nc.vector.memset(sent_row[:], 0.0)
            nc.vector.memset(sent_row[:, HC:HC + H], SENT_AS)

            # ---------------- phase A1: hx1 tables = x @ W1ext -------------
            with (
                tc.tile_pool(name="pa_sbuf", bufs=3) as spool,
                tc.tile_pool(name="pa_out", bufs=3) as opool,
                tc.tile_pool(name="pa_psum", bufs=4, space="PSUM") as pps,
            ):
                for t0 in range(0, nt1, TB):
                    tb = min(TB, nt1 - t0)
                    xt = spool.tile([P, 2, TB * P], F16, tag="xt")
                    for kk in range(2):
                        nc.sync.dma_start(
                            out=xt[:, kk, :tb * P],
                            in_=xT_ap[kk, :, t0 * P:(t0 + tb) * P])
                    stage = opool.tile([P, TB, RW1], F16, tag="stage")
                    for ti in range(tb):
                        ps = pps.tile([P, RW1], F32, tag="ps")
                        for kk in range(2):
                            nc.tensor.matmul(
                                ps[:], lhsT=xt[:, kk, ti * P:(ti + 1) * P],
                                rhs=w1_sb[:, kk, :],
                                start=(kk == 0), stop=(kk == 1))
                        if ti % 2 == 0:
                            nc.scalar.copy(stage[:, ti, :], ps[:])
                        else:
                            nc.vector.tensor_copy(stage[:, ti, :], ps[:])
                    # route tiles to the lo/hi tables
                    spans = []
                    if t0 < lo_tiles:
                        n_lo = min(tb, lo_tiles - t0)
                        spans.append((hx1_lo, t0 * P, 0, n_lo))
                        if n_lo < tb:
                            spans.append((hx1_hi, 0, n_lo, tb - n_lo))
                    else:
                        spans.append((hx1_hi, t0 * P - losplit, 0, tb))
                    for tab, r0, ti0, ntl in spans:
                        nc.sync.dma_start(
                            out=tab[r0:r0 + ntl * P, :RW1].rearrange(
                                "(t p) w -> p t w", p=P),
                            in_=stage[:, ti0:ti0 + ntl, :])
                    nc.sync.dma_start(
                        out=ad1[t0 * P:(t0 + tb) * P, :].rearrange(
                            "(t p) w -> p t w", p=P),
                        in_=stage[:, :tb, PAY1:PAY1 + H])
                # sentinel rows: full zero row with as=-30000 in lo; the hi
                # sentinel (a real pad row, h already 0) gets just as cols
                nc.sync.dma_start(out=hx1_lo[lo_sent:lo_sent + 1, :RW1],
                                  in_=sent_row[0:1, :])
                nc.sync.dma_start(
                    out=hx1_hi[hi_sent:hi_sent + 1, HC:HC + H],
                    in_=sent_row[0:1, HC:HC + H])

            # ---------------- phase B1: layer-1 aggregation + W2 fold ------
            with (
                tc.tile_pool(name="pb_gather", bufs=3) as gpool,
                tc.tile_pool(name="pb_work", bufs=3) as wpool,
                tc.tile_pool(name="pb_ep", bufs=2) as epool,
                tc.tile_pool(name="pb_psum", bufs=2, space="PSUM") as upps,
                tc.tile_pool(name="pb_psumT", bufs=2, space="PSUM") as tpps,
            ):
                for g in range(ngroups):
                    adg_w = epool.tile([P, TRW2], F16, tag="adg_w")
                    nc.gpsimd.indirect_dma_start(
                        out=adg_w[:, :H], out_offset=None, in_=ad1,
                        in_offset=bass.IndirectOffsetOnAxis(
                            ap=nodei[:, g:g + 1], axis=0))
                    nc.sync.dma_start(
                        out=ad1_loc[g * P:(g + 1) * P, :H], in_=adg_w[:, :H])
                for g in range(ngroups):
                    pay = gpool.tile([P, ebt1, TRW1], F16, tag="pay")
                    nc.gpsimd.dma_gather(
                        out_ap=pay[:, :e1lo, :],
                        in_ap=hx1_lo[:],
                        idxs_ap=lo1_sb[:, g * e1lo * 8:(g + 1) * e1lo * 8],
                        num_idxs=e1lo * P, num_idxs_reg=e1lo * P,
                        elem_size=TRW1, single_packet=False)
                    nc.gpsimd.dma_gather(
                        out_ap=pay[:, e1lo:, :],
                        in_ap=hx1_hi[:],
                        idxs_ap=hi1_sb[:, g * e1hi * 8:(g + 1) * e1hi * 8],
                        num_idxs=e1hi * P, num_idxs_reg=e1hi * P,
                        elem_size=TRW1, single_packet=False)
                    ade = gpool.tile([P, ebt1, TRW2], F16, tag="ade")
                    nc.gpsimd.dma_gather(
                        out_ap=ade[:],
                        in_ap=ad1_loc[:],
                        idxs_ap=de1_sb[:, g * ebt1 * 8:(g + 1) * ebt1 * 8],
                        num_idxs=ebt1 * P, num_idxs_reg=ebt1 * P,
                        elem_size=TRW2, single_packet=False)
                    cs1 = slice(g * ebt1, (g + 1) * ebt1)
                    msb = wpool.tile([P, ebt1, P], F16, tag="msb")
                    nc.vector.tensor_tensor(
                        out=msb[:],
                        in0=iota_f[:, None, :].to_broadcast([P, ebt1, P]),
                        in1=dstl1[:, cs1][:, :, None].to_broadcast(
                            [P, ebt1, P]),
                        op=mybir.AluOpType.is_equal)
                    z = wpool.tile([P, ebt1, H], F32, tag="z")
                    nc.vector.tensor_tensor(
                        out=z[:], in0=pay[:, :, HC:HC + H],
                        in1=ade[:, :, :H],
                        op=mybir.AluOpType.add)
                    z2 = wpool.tile([P, ebt1, H], F32, tag="z2")
                    nc.vector.tensor_scalar_mul(z2[:], z[:], 0.2)
                    lr = wpool.tile([P, ebt1, H], F32, tag="lr")
                    nc.vector.tensor_tensor(out=lr[:], in0=z[:], in1=z2[:],
                                            op=mybir.AluOpType.max)
                    gsb = wpool.tile([P, ebt1, PAY1], F16, tag="gsb")
                    nc.scalar.activation(
                        out=gsb[:, :, HC:], in_=lr[:],
                        func=mybir.ActivationFunctionType.Exp,
                        bias=nshift[:])
                    nc.vector.tensor_tensor(
                        out=gsb[:, :, :HC].rearrange(
                            "p j (c h) -> p j c h", h=H),
                        in0=pay[:, :, :HC].rearrange(
                            "p j (c h) -> p j c h", h=H),
                        in1=gsb[:, :, None, HC:].to_broadcast([P, ebt1, CH, H]),
                        op=mybir.AluOpType.mult)
                    u_ps = upps.tile([P, PAY1], F32, tag="u_ps")
                    for j in range(ebt1):
                        nc.tensor.matmul(u_ps[:], lhsT=msb[:, j, :],
                                         rhs=gsb[:, j, :],
                                         start=(j == 0), stop=(j == ebt1 - 1))
                    # epilogue
                    s_sb = epool.tile([P, H], F32, tag="s_sb")
                    nc.vector.tensor_scalar_add(s_sb[:], u_ps[:, HC:], 1e-16)
                    r_sb = epool.tile([P, H], F32, tag="r_sb")
                    nc.vector.reciprocal(r_sb[:], s_sb[:])
                    zt = epool.tile([P, HC], F32, tag="zt")
                    nc.vector.tensor_tensor(
                        out=zt[:].rearrange("p (c h) -> p c h", h=H),
                        in0=u_ps[:, :HC].rearrange("p (c h) -> p c h", h=H),
                        in1=r_sb[:][:, None, :].to_broadcast([P, CH, H]),
                        op=mybir.AluOpType.mult)
                    zb = epool.tile([P, HC], F16, tag="zb")
                    nc.vector.tensor_tensor(out=zb[:], in0=zt[:], in1=b1bc[:],
                                            op=mybir.AluOpType.add)
                    t1 = epool.tile([P, HC], F16, tag="t1")
                    nc.vector.tensor_scalar(out=t1[:], in0=zb[:], scalar1=0.0,
                                            scalar2=None,
                                            op0=mybir.AluOpType.min)
                    t2 = epool.tile([P, HC], F16, tag="t2")
                    nc.scalar.activation(out=t2[:], in_=t1[:],
                                         func=mybir.ActivationFunctionType.Exp)
                    t3 = epool.tile([P, HC], F16, tag="t3")
                    nc.vector.tensor_scalar_add(t3[:], t2[:], -1.0)
                    h16 = epool.tile([P, HC], F16, tag="h16")
                    nc.vector.tensor_tensor(out=h16[:], in0=zb[:], in1=t3[:],
                                            op=mybir.AluOpType.max)
                    h2_ps = tpps.tile([P, RW2], F32, tag="h2_ps")
                    for kk in range(2):
                        hT_ps = tpps.tile([P, P], F16, tag="hT_ps")
                        nc.tensor.transpose(hT_ps[:],
                                            h16[:, kk * P:(kk + 1) * P],
                                            ident16[:])
                        hT_sb = epool.tile([P, P], F16, tag="hT_sb")
                        nc.vector.tensor_copy(hT_sb[:], hT_ps[:])
                        nc.tensor.matmul(h2_ps[:], lhsT=hT_sb[:],
                                         rhs=w2_sb[:, kk, :],
                                         start=(kk == 0), stop=(kk == 1))
                    nc.scalar.copy(hx2_sb[:, g, :], h2_ps[:])
                    if split_cc and g == gh - 1:
                        # first-half allgather overlaps the remaining groups
                        nc.sync.dma_start(
                            out=cc_in[:gh * P].rearrange(
                                "(g p) w -> p g w", p=P),
                            in_=hx2_sb[:, :gh, :])
                        nc.gpsimd.collective_compute(
                            "AllGather", mybir.AluOpType.bypass,
                            replica_groups=[list(range(n_cores))],
                            ins=[cc_in[:gh * P]], outs=[hx2f_a[:]])
                if not split_cc:
                    nc.sync.dma_start(
                        out=cc_in[:gh * P].rearrange("(g p) w -> p g w", p=P),
                        in_=hx2_sb[:, :gh, :])
                    nc.gpsimd.collective_compute(
                        "AllGather", mybir.AluOpType.bypass,
                        replica_groups=[list(range(n_cores))],
                        ins=[cc_in[:gh * P]], outs=[hx2f_a[:]])
                nc.sync.dma_start(
                    out=cc_in[gh * P:].rearrange("(g p) w -> p g w", p=P),
                    in_=hx2_sb[:, gh:, :])
                # layer-2 sentinel: as2 = -30000 on the first pad row
                assert sent2 >= gh * P
                nc.sync.dma_start(
                    out=cc_in[sent2:sent2 + 1, PAY2 - 1:PAY2],
                    in_=sent_row[0:1, HC:HC + 1])
                nc.gpsimd.collective_compute(
                    "AllGather", mybir.AluOpType.bypass,
                    replica_groups=[list(range(n_cores))],
                    ins=[cc_in[gh * P:]], outs=[hx2f_b[:]])
                nc.sync.dma_start(
                    out=ad2_loc[:, :1].rearrange("(g p) w -> p g w", p=P),
                    in_=hx2_sb[:, :, PAY2:PAY2 + 1])

            # ------------- expand hx2f into padded lo/hi tables ------------
            with tc.tile_pool(name="px", bufs=4) as xpool:
                for r in range(n_cores):
                    xt2 = xpool.tile([P, gh, RW2], F16, tag="xt2")
                    nc.sync.dma_start(
                        out=xt2[:],
                        in_=hx2f_a[r].rearrange("(g p) w -> p g w", p=P))
                    tab = hx2p_lo if r < halfc else hx2p_hi
                    r0 = (r % halfc) * npc_pad
                    nc.sync.dma_start(
                        out=tab[r0:r0 + gh * P, :PAY2 + 1].rearrange(
                            "(g p) w -> p g w", p=P),
                        in_=xt2[:, :, :PAY2 + 1])
                for r in range(n_cores):
                    xt3 = xpool.tile([P, ngroups - gh, RW2], F16, tag="xt3")
                    nc.sync.dma_start(
                        out=xt3[:],
                        in_=hx2f_b[r].rearrange("(g p) w -> p g w", p=P))
                    tab = hx2p_lo if r < halfc else hx2p_hi
                    r0 = (r % halfc) * npc_pad
                    nc.sync.dma_start(
                        out=tab[r0 + gh * P:r0 + npc_pad, :PAY2 + 1].rearrange(
                            "(g p) w -> p g w", p=P),
                        in_=xt3[:, :, :PAY2 + 1])
                if debug_outputs:
                    dt2 = xpool.tile([P, ngroups, RW2], F16, tag="dt2")
                    nc.sync.dma_start(
                        out=dt2[:],
                        in_=cc_in[:].rearrange("(g p) w -> p g w", p=P))
                    nc.sync.dma_start(
                        out=dbg_cc[:].rearrange("(g p) w -> p g w", p=P),
                        in_=dt2[:])

            # ---------------- phase B2: layer-2 aggregation ----------------
            with (
                tc.tile_pool(name="p2_gather", bufs=3) as g2pool,
                tc.tile_pool(name="p2_work", bufs=3) as w2pool,
                tc.tile_pool(name="p2_ep", bufs=2) as e2pool,
                tc.tile_pool(name="p2_psum", bufs=2, space="PSUM") as u2ps,
            ):
                for g in range(ngroups):
                    pay = g2pool.tile([P, ebt2, TRW2], F16, tag="pay2")
                    nc.gpsimd.dma_gather(
                        out_ap=pay[:, :e2lo, :],
                        in_ap=hx2p_lo[:],
                        idxs_ap=lo2_sb[:, g * e2lo * 8:(g + 1) * e2lo * 8],
                        num_idxs=e2lo * P, num_idxs_reg=e2lo * P,
                        elem_size=TRW2, single_packet=False)
                    nc.gpsimd.dma_gather(
                        out_ap=pay[:, e2lo:, :],
                        in_ap=hx2p_hi[:],
                        idxs_ap=hi2_sb[:, g * e2hi * 8:(g + 1) * e2hi * 8],
                        num_idxs=e2hi * P, num_idxs_reg=e2hi * P,
                        elem_size=TRW2, single_packet=False)
                    ade = g2pool.tile([P, ebt2, TRW2], F16, tag="ade2")
                    nc.gpsimd.dma_gather(
                        out_ap=ade[:],
                        in_ap=ad2_loc[:],
                        idxs_ap=de2_sb[:, g * ebt2 * 8:(g + 1) * ebt2 * 8],
                        num_idxs=ebt2 * P, num_idxs_reg=ebt2 * P,
                        elem_size=TRW2, single_packet=False)
                    cs2 = slice(g * ebt2, (g + 1) * ebt2)
                    msb = w2pool.tile([P, ebt2, P], F16, tag="msb2")
                    nc.vector.tensor_tensor(
                        out=msb[:],
                        in0=iota_f[:, None, :].to_broadcast([P, ebt2, P]),
                        in1=dstl2[:, cs2][:, :, None].to_broadcast(
                            [P, ebt2, P]),
                        op=mybir.AluOpType.is_equal)
                    z = w2pool.tile([P, ebt2, 1], F32, tag="z")
                    nc.vector.tensor_tensor(out=z[:],
                                            in0=pay[:, :, OUT:OUT + 1],
                                            in1=ade[:, :, :1],
                                            op=mybir.AluOpType.add)
                    z2 = w2pool.tile([P, ebt2, 1], F32, tag="z2")
                    nc.vector.tensor_scalar_mul(z2[:], z[:], 0.2)
                    lr = w2pool.tile([P, ebt2, 1], F32, tag="lr")
                    nc.vector.tensor_tensor(out=lr[:], in0=z[:], in1=z2[:],
                                            op=mybir.AluOpType.max)
                    gsb = w2pool.tile([P, ebt2, PAY2], F16, tag="gsb2")
                    nc.scalar.activation(
                        out=gsb[:, :, OUT:], in_=lr[:],
                        func=mybir.ActivationFunctionType.Exp, bias=0.0)
                    nc.vector.tensor_tensor(
                        out=gsb[:, :, :OUT],
                        in0=pay[:, :, :OUT],
                        in1=gsb[:, :, OUT:].to_broadcast([P, ebt2, OUT]),
                        op=mybir.AluOpType.mult)
                    u_ps = u2ps.tile([P, PAY2], F32, tag="u_ps2")
                    for j in range(ebt2):
                        nc.tensor.matmul(u_ps[:], lhsT=msb[:, j, :],
                                         rhs=gsb[:, j, :],
                                         start=(j == 0), stop=(j == ebt2 - 1))
                    s_sb = e2pool.tile([P, 1], F32, tag="s_sb2")
                    nc.vector.tensor_scalar_add(s_sb[:], u_ps[:, OUT:], 1e-16)
                    r_sb = e2pool.tile([P, 1], F32, tag="r_sb2")
                    nc.vector.reciprocal(r_sb[:], s_sb[:])
                    y_sb = e2pool.tile([P, OUT], F32, tag="y_sb")
                    nc.scalar.activation(
                        out=y_sb[:], in_=u_ps[:, :OUT],
                        func=mybir.ActivationFunctionType.Copy,
                        scale=r_sb[:, 0:1])
                    yb = e2pool.tile([P, OUT], F32, tag="yb")
                    nc.vector.tensor_tensor(out=yb[:], in0=y_sb[:],
                                            in1=b2bc[:],
                                            op=mybir.AluOpType.add)
                    nc.sync.dma_start(out=y_ap[g * P:(g + 1) * P, :],
                                      in_=yb[:])
    nc.compile()
    return nc


# --------------------------------------------------------------------------
# host-side input prep
# --------------------------------------------------------------------------

def prep_inputs(inputs, cfg, tables):
    x = np.asarray(inputs["x"], np.float32)
    npad1 = cfg["npad1"]
    xT = np.zeros((D, npad1), np.float16)
    xT[:, :cfg["n_valid"]] = x.T.astype(np.float16)
    xT16 = np.ascontiguousarray(xT.reshape(2, P, npad1))
    W1ext, W2ext = fold_weights(
        inputs["W1"], inputs["a_src1"], inputs["a_dst1"],
        inputs["W2"], inputs["a_src2"], inputs["a_dst2"])
    b1 = np.asarray(inputs["b1"], np.float32)
    b2 = np.asarray(inputs["b2"], np.float32)
    in_maps = [dict(
        xT16=xT16, w1ext=W1ext, b1=b1, w2ext=W2ext, b2=b2,
        lo1=t["lo1"], hi1=t["hi1"], dstl1=t["dstl1"], dste1=t["dste1"],
        lo2=t["lo2"], hi2=t["hi2"], dstl2=t["dstl2"], dste2=t["dste2"],
        nodeidx=t["nodeidx"],
    ) for t in tables]
    return in_maps


_CACHE = {}


import os as _os


def get_nc(cfg, ebs):
    split = _os.environ.get("K_SPLIT_CC", "1") == "1"
    key = (cfg["n_valid"], cfg["n_cores"], ebs, split)
    if key not in _CACHE:
        _CACHE[key] = build_fused(cfg, ebs, split_cc=split)
    return _CACHE[key]


def _run_with_retry(nc, in_maps, tries=3):
    from concourse.bass_utils import run_bass_kernel_spmd
    last = None
    for attempt in range(tries):
        try:
            return run_bass_kernel_spmd(nc, in_maps,
                                        core_ids=list(range(len(in_maps))))
        except Exception as e:  # noqa: BLE001 - retry any runtime failure
            last = e
            import time as _time
            _time.sleep(2.0 * (attempt + 1))
    raise last


def kernel(**inputs):
    """Full-input GAT kernel on 8 Trainium2 NeuronCores.

    Takes the unsharded inputs of reference.setup_inputs(), distributes the
    work across 8 cores (dst-node graph partition) in a single fused launch
    with an on-device AllGather between the layers, and returns the full
    [50000, 32] float32 output.
    """
    x = np.asarray(inputs["x"], np.float32)
    ei = np.asarray(inputs["edge_index"])
    N = x.shape[0]
    cfg = make_cfg(N)
    src = np.concatenate([ei[0].astype(np.int64),
                          np.arange(N, dtype=np.int64)])
    dst = np.concatenate([ei[1].astype(np.int64),
                          np.arange(N, dtype=np.int64)])
    tables, ebs = build_edge_tables(src, dst, cfg)
    nc = get_nc(cfg, ebs)
    in_maps = prep_inputs(inputs, cfg, tables)
    res = _run_with_retry(nc, in_maps)
    npc = cfg["npc"]
    y = np.concatenate([res.results[k]["y"][:npc]
                        for k in range(cfg["n_cores"])], axis=0)
    return y.astype(np.float32)


# revision 17
# speedup vs baseline: 1.3425x; 1.2918x over previous
"""Two-layer GAT on 8 Trainium2 cores via Bass/Tile — fused single launch.

Strategy (dst-node graph partition, per the sharding hint):
- Nodes are split into 8 contiguous ranges (6250 per core); every edge is
  owned by the core that owns its dst node.
- Single launch per core:
  * Phase A1 (redundant on every core): hx1 = x @ [W1p | W1@Asrc] written as
    fp16 tables with 768B rows [h (c,h-interleaved) 256 | a_src.h 8 | pad],
    split lo/hi at row 32512 so dma_gather's int16 indices stay in range;
    a_dst.h goes to a compact ad1 [npad1, 8] side table. Sentinel rows have
    a_src = -30000 (dummy edges gather them; exp == 0).
  * Phase B1 (layer-1 edge aggregation, per 128-dst-node group):
    two dma_gathers (lo/hi) fetch all edge payload rows [128, EBT, 384].
    The group's a_dst block adg [128, 8] comes from one plain strided DMA
    (dst nodes of a group are contiguous); per-edge a_dst is M_T @ adg on
    the PE, where M_T (and M for the scatter) are built by two batched
    is_equal ops from the dst-offset table. Batched DVE/Act ops compute
    w = exp(lrelu(as+ad) - 6) and G = [h*w | w]; EBT PE matmuls accumulate
    U += M_j.T @ G_j in PSUM. Epilogue: h1 = elu(U/(sum)+b1), transposed on
    PE and folded through W2 immediately: hx2 row = [h1@W2 | a2s.h1 | a2d.h1]
    into an SBUF shard table (the layer-2 dense phase is fused in here).
  * AllGather (on-device collective) of the per-core hx2 shard [6272, 36]
    -> [8, 6272, 36] Shared DRAM, then expanded on-device into two padded
    256B-row tables (ranks 0-3 / 4-7) for int16 dma_gather indexing.
  * Phase B2: per-group aggregation as B1 (adg2 is already in SBUF from the
    fold); writes y rows [6272, 32] f32.
- Host: slices the 8 y shards to 6250 rows each and concatenates.
"""

import sys
for _p in ("/opt/trn_rl_repo",):
    if _p not in sys.path:
        sys.path.append(_p)

import math
import numpy as np

import concourse.bass as bass
import concourse.mybir as mybir
import concourse.tile as tile
from concourse import bacc
from concourse.masks import make_identity

F32 = mybir.dt.float32
F16 = mybir.dt.float16
I16 = mybir.dt.int16

N_CORES = 8
D = 256
HC = 256
H = 8
CH = 32
OUT = 32
P = 128
TRW1 = 384       # hx1 table row width (fp16) = 768B (dma_gather granularity)
RW1 = 272        # written row prefix: 256 h + 8 as + 8 ad
PAY1 = 264
TRW2 = 128       # hx2 padded table row width (fp16) = 256B
RW2 = 36         # compact hx2 row: 32 h2 + 1 as2 + 1 ad2 + 2 pad
PAY2 = 33
SHIFT1 = 6.0
SENT_AS = -30000.0
LOSPLIT1 = 32512  # 254 * 128


def make_cfg(n_valid, n_cores=N_CORES):
    npc = n_valid // n_cores
    assert npc * n_cores == n_valid
    ngroups = math.ceil(npc / P)
    npc_pad = ngroups * P
    assert npc_pad > npc, "need a pad row for the layer-2 sentinel"
    nt1 = math.ceil(n_valid / P)
    npad1 = nt1 * P
    losplit = min(LOSPLIT1, (nt1 // 2) * P)
    lo_rows = losplit + P             # + sentinel row block
    hi_rows = npad1 - losplit
    assert losplit <= 32767 and hi_rows <= 32767
    half = (n_cores // 2) * npc_pad   # L2 lo/hi split (remapped rows)
    assert half <= 32767 and n_cores * npc_pad - half <= 32767
    return dict(
        n_valid=n_valid, n_cores=n_cores, npc=npc, ngroups=ngroups,
        npc_pad=npc_pad, nt1=nt1, npad1=npad1,
        losplit=losplit, lo_rows=lo_rows, hi_rows=hi_rows,
        lo_sent=losplit, hi_sent=n_valid - losplit,
        half2=half, sent2=npc,
    )


def fold_weights(W1, a_src1, a_dst1, W2, a_src2, a_dst2):
    W1 = np.asarray(W1, np.float32)
    a_src1 = np.asarray(a_src1, np.float32)
    a_dst1 = np.asarray(a_dst1, np.float32)
    A_src = np.zeros((HC, H), np.float32)
    A_dst = np.zeros((HC, H), np.float32)
    for h in range(H):
        A_src[h * CH:(h + 1) * CH, h] = a_src1[h]
        A_dst[h * CH:(h + 1) * CH, h] = a_dst1[h]
    # interleave the h-block columns to (c, h) order so DVE broadcast muls
    # on-device have a packed last dim
    perm = (np.arange(CH)[:, None] + np.arange(H)[None, :] * CH).reshape(-1)
    W1ext = np.concatenate([W1[:, perm], W1 @ A_src, W1 @ A_dst], axis=1)
    W2 = np.asarray(W2, np.float32)
    W2e = np.concatenate(
        [W2, W2 @ np.asarray(a_src2, np.float32).T,
         W2 @ np.asarray(a_dst2, np.float32).T,
         np.zeros((HC, RW2 - PAY2 - 1), np.float32)], axis=1)   # [256, 36]
    # rows of W2e must match the (c,h)-permuted h1 produced by the PE
    # transposes: contraction index kk*128 + (c - 16*kk)*8 + h <-> row h*32+c
    rperm = np.empty(HC, np.int64)
    i = 0
    for kk in range(2):
        for cl in range(16):
            for h in range(H):
                rperm[i] = h * CH + kk * 16 + cl
                i += 1
    W2ext = W2e[rperm]
    return W1ext.astype(np.float16), W2ext.astype(np.float16)


def _wrap16(arr):
    # dma_gather index layout: ordinal i -> [i % 16, i // 16], x8 rows
    n = arr.size
    return np.tile(arr.reshape(n // 16, 16).T, (8, 1)).astype(np.int16)


def _bucket_tables(src_rows, dst_local, grp, ngroups, losplit, lo_sent,
                   hi_sent, eb_lo, eb_hi):
    """Bucket one core's edges into per-group lo/hi blocks.

    src_rows: table row index per edge; dst_local: dst offset within group
    [0,128); grp: group id per edge. Returns (lo_idx, hi_idx, dstl) with
    lo_idx/hi_idx int16-wrapped [128, ngroups*eb*8] and dstl f16
    [128, ngroups*(eb_lo+eb_hi)].
    """
    islo_all = src_rows < losplit
    ebt = eb_lo + eb_hi
    lo_a = np.full((ngroups, eb_lo * P), lo_sent, np.int64)
    hi_a = np.full((ngroups, eb_hi * P), hi_sent, np.int64)
    dste_a = np.zeros((ngroups, ebt * P), np.int64)
    dstl = np.zeros((P, ngroups * ebt), np.float16)
    order = np.argsort(grp, kind="stable")
    src_rows, dst_local, grp, islo_all = (src_rows[order], dst_local[order],
                                          grp[order], islo_all[order])
    gstart = np.searchsorted(grp, np.arange(ngroups + 1))
    for g in range(ngroups):
        a, b = gstart[g], gstart[g + 1]
        sl, dl, lo = src_rows[a:b], dst_local[a:b], islo_all[a:b]
        nlo = int(lo.sum())
        nhi = (b - a) - nlo
        assert nlo <= eb_lo * P and nhi <= eb_hi * P
        lo_a[g, :nlo] = sl[lo]
        hi_a[g, :nhi] = sl[~lo] - losplit
        ilo = np.arange(nlo)
        dstl[ilo % P, g * ebt + ilo // P] = dl[lo].astype(np.float16)
        dste_a[g, :nlo] = g * P + dl[lo]
        ihi = np.arange(nhi)
        dstl[ihi % P, g * ebt + eb_lo + ihi // P] = dl[~lo].astype(np.float16)
        dste_a[g, eb_lo * P:eb_lo * P + nhi] = g * P + dl[~lo]
    lo_idx = np.concatenate([_wrap16(lo_a[g]) for g in range(ngroups)], axis=1)
    hi_idx = np.concatenate([_wrap16(hi_a[g]) for g in range(ngroups)], axis=1)
    dste = np.concatenate([_wrap16(dste_a[g]) for g in range(ngroups)], axis=1)
    return lo_idx, hi_idx, dstl, dste


def build_edge_tables(src, dst, cfg, ebs=None):
    """Per-core gather tables for both layers. ebs = (eb1_lo, eb1_hi,
    eb2_lo, eb2_hi) or None to size from the data."""
    npc, ngroups = cfg["npc"], cfg["ngroups"]
    npc_pad = cfg["npc_pad"]
    losplit, lo_sent, hi_sent = cfg["losplit"], cfg["lo_sent"], cfg["hi_sent"]
    half2, sent2 = cfg["half2"], cfg["sent2"]
    n_cores = cfg["n_cores"]

    src = np.asarray(src, np.int64)
    dst = np.asarray(dst, np.int64)
    core = dst // npc

    per_core = []
    for k in range(n_cores):
        m = core == k
        s_k = src[m]
        d_k = dst[m] - k * npc
        g_k = d_k // P
        dl_k = d_k - g_k * P
        s2_k = (s_k // npc) * npc_pad + (s_k % npc)   # remapped L2 rows
        per_core.append((s_k, s2_k, dl_k, g_k))

    if ebs is not None:
        e1lo, e1hi, e2lo, e2hi = ebs
    else:
        e1lo = e1hi = e2lo = e2hi = 1
        for s_k, s2_k, dl_k, g_k in per_core:
            islo1 = s_k < losplit
            c = np.bincount(g_k[islo1], minlength=ngroups).max()
            e1lo = max(e1lo, math.ceil(c / P))
            c = np.bincount(g_k[~islo1], minlength=ngroups).max()
            e1hi = max(e1hi, math.ceil(c / P))
            islo2 = s2_k < half2
            c = np.bincount(g_k[islo2], minlength=ngroups).max()
            e2lo = max(e2lo, math.ceil(c / P))
            c = np.bincount(g_k[~islo2], minlength=ngroups).max()
            e2hi = max(e2hi, math.ceil(c / P))

    tables = []
    for s_k, s2_k, dl_k, g_k in per_core:
        lo1, hi1, dstl1, dste1 = _bucket_tables(
            s_k, dl_k, g_k, ngroups, losplit, lo_sent, hi_sent, e1lo, e1hi)
        # L2 sentinel: every rank's local pad row `sent2` has as2=-30000;
        # lo dummies -> rank0's (row sent2), hi dummies -> rank halfc's
        # (local row sent2 within the hi table)
        lo2, hi2, dstl2, dste2 = _bucket_tables(
            s2_k, dl_k, g_k, ngroups, half2, sent2, sent2, e2lo, e2hi)
        tables.append(dict(lo1=lo1, hi1=hi1, dstl1=dstl1, dste1=dste1,
                           lo2=lo2, hi2=hi2, dstl2=dstl2, dste2=dste2))
    npad1 = cfg["npad1"]
    for k, t in enumerate(tables):
        ni = (k * npc + np.arange(P)[:, None]
              + P * np.arange(ngroups)[None, :])
        t["nodeidx"] = np.minimum(ni, npad1 - 1).astype(np.int32)
    return tables, (e1lo, e1hi, e2lo, e2hi)


# --------------------------------------------------------------------------
# fused launch
# --------------------------------------------------------------------------

def build_fused(cfg, ebs, num_devices=N_CORES, debug_outputs=False,
                split_cc=True):
    npc, ngroups = cfg["npc"], cfg["ngroups"]
    npc_pad, nt1, npad1 = cfg["npc_pad"], cfg["nt1"], cfg["npad1"]
    losplit, lo_rows, hi_rows = cfg["losplit"], cfg["lo_rows"], cfg["hi_rows"]
    lo_sent, hi_sent = cfg["lo_sent"], cfg["hi_sent"]
    sent2 = cfg["sent2"]
    n_cores = cfg["n_cores"]
    e1lo, e1hi, e2lo, e2hi = ebs
    ebt1, ebt2 = e1lo + e1hi, e2lo + e2hi
    lo_tiles = losplit // P
    halfc = n_cores // 2

    nc = bacc.Bacc("TRN2", target_bir_lowering=False, debug=False,
                   num_devices=num_devices)
    xT_ap = nc.dram_tensor("xT16", [2, P, npad1], F16, kind="ExternalInput").ap()
    w1_ap = nc.dram_tensor("w1ext", [D, RW1], F16, kind="ExternalInput").ap()
    b1_ap = nc.dram_tensor("b1", [HC], F32, kind="ExternalInput").ap()
    w2_ap = nc.dram_tensor("w2ext", [HC, RW2], F16, kind="ExternalInput").ap()
    b2_ap = nc.dram_tensor("b2", [OUT], F32, kind="ExternalInput").ap()
    lo1_ap = nc.dram_tensor("lo1", [P, ngroups * e1lo * 8], I16,
                            kind="ExternalInput").ap()
    hi1_ap = nc.dram_tensor("hi1", [P, ngroups * e1hi * 8], I16,
                            kind="ExternalInput").ap()
    dstl1_ap = nc.dram_tensor("dstl1", [P, ngroups * ebt1], F16,
                              kind="ExternalInput").ap()
    lo2_ap = nc.dram_tensor("lo2", [P, ngroups * e2lo * 8], I16,
                            kind="ExternalInput").ap()
    hi2_ap = nc.dram_tensor("hi2", [P, ngroups * e2hi * 8], I16,
                            kind="ExternalInput").ap()
    dstl2_ap = nc.dram_tensor("dstl2", [P, ngroups * ebt2], F16,
                              kind="ExternalInput").ap()
    ni_ap = nc.dram_tensor("nodeidx", [P, ngroups], mybir.dt.int32,
                           kind="ExternalInput").ap()

    y_ap = nc.dram_tensor("y", [npc_pad, OUT], F32, kind="ExternalOutput").ap()

    hx1_lo = nc.dram_tensor("hx1_lo", [lo_rows, TRW1], F16).ap()
    hx1_hi = nc.dram_tensor("hx1_hi", [hi_rows, TRW1], F16).ap()
    ad1 = nc.dram_tensor("ad1", [npad1, H], F16).ap()
    gh = ngroups // 2
    cc_in = nc.dram_tensor("cc_in", [npc_pad, RW2], F16).ap()
    hx2f_a = nc.dram_tensor("hx2f_a", [n_cores, gh * P, RW2], F16,
                            addr_space="Shared").ap()
    hx2f_b = nc.dram_tensor("hx2f_b", [n_cores, (ngroups - gh) * P, RW2],
                            F16, addr_space="Shared").ap()
    hx2p_lo = nc.dram_tensor("hx2p_lo", [halfc * npc_pad, TRW2], F16).ap()
    hx2p_hi = nc.dram_tensor("hx2p_hi", [halfc * npc_pad, TRW2], F16).ap()
    if debug_outputs:
        dbg_cc = nc.dram_tensor("dbg_cc", [npc_pad, RW2], F16,
                                kind="ExternalOutput").ap()

    TB = 8

    with tile.TileContext(nc) as tc:
        with tc.tile_pool(name="const", bufs=1) as cpool:
            ident16 = cpool.tile([P, P], F16)
            make_identity(nc, ident16[:])
            iota_f = cpool.tile([P, P], F16)
            nc.gpsimd.iota(iota_f[:], pattern=[[1, P]], base=0,
                           channel_multiplier=0,
                           allow_small_or_imprecise_dtypes=True)
            w1_sb = cpool.tile([P, 2, RW1], F16)
            nc.sync.dma_start(out=w1_sb[:, 0, :], in_=w1_ap[0:P, :])
            nc.sync.dma_start(out=w1_sb[:, 1, :], in_=w1_ap[P:2 * P, :])
            w2_sb = cpool.tile([P, 2, RW2], F16)
            nc.sync.dma_start(out=w2_sb[:, 0, :], in_=w2_ap[0:P, :])
            nc.sync.dma_start(out=w2_sb[:, 1, :], in_=w2_ap[P:2 * P, :])
            b1bc = cpool.tile([P, HC], F32)
            for hh in range(H):
                nc.sync.dma_start(
                    out=b1bc[:].rearrange("p (c h) -> p c h", h=H)[:, :, hh],
                    in_=b1_ap[hh * CH:(hh + 1) * CH][None, :]
                    .to_broadcast([P, CH]))
            b2bc = cpool.tile([P, OUT], F32)
            nc.sync.dma_start(out=b2bc[:],
                              in_=b2_ap[None, :].to_broadcast([P, OUT]))
            lo1_sb = cpool.tile([P, ngroups * e1lo * 8], I16)
            nc.sync.dma_start(out=lo1_sb[:], in_=lo1_ap[:])
            hi1_sb = cpool.tile([P, ngroups * e1hi * 8], I16)
            nc.sync.dma_start(out=hi1_sb[:], in_=hi1_ap[:])
            dstl1 = cpool.tile([P, ngroups * ebt1], F16)
            nc.sync.dma_start(out=dstl1[:], in_=dstl1_ap[:])
            lo2_sb = cpool.tile([P, ngroups * e2lo * 8], I16)
            nc.sync.dma_start(out=lo2_sb[:], in_=lo2_ap[:])
            hi2_sb = cpool.tile([P, ngroups * e2hi * 8], I16)
            nc.sync.dma_start(out=hi2_sb[:], in_=hi2_ap[:])
            dstl2 = cpool.tile([P, ngroups * ebt2], F16)
            nc.sync.dma_start(out=dstl2[:], in_=dstl2_ap[:])
            nodei = cpool.tile([P, ngroups], mybir.dt.int32)
            nc.sync.dma_start(out=nodei[:], in_=ni_ap[:])

            hx2_sb = cpool.tile([P, ngroups, RW2], F16)
            nshift = cpool.tile([P, 1], F32)
            nc.gpsimd.memset(nshift[:], -SHIFT1)
            sent_row = cpool.tile([P, RW1], F16)
            nc.vector.memset(sent_row[:], 0.0)
            nc.vector.memset(sent_row[:, HC:HC + H], SENT_AS)

            # ---------------- phase A1: hx1 tables = x @ W1ext -------------
            with (
                tc.tile_pool(name="pa_sbuf", bufs=3) as spool,
                tc.tile_pool(name="pa_out", bufs=3) as opool,
                tc.tile_pool(name="pa_psum", bufs=4, space="PSUM") as pps,
            ):
                for t0 in range(0, nt1, TB):
                    tb = min(TB, nt1 - t0)
                    xt = spool.tile([P, 2, TB * P], F16, tag="xt")
                    for kk in range(2):
                        nc.sync.dma_start(
                            out=xt[:, kk, :tb * P],
                            in_=xT_ap[kk, :, t0 * P:(t0 + tb) * P])
                    stage = opool.tile([P, TB, RW1], F16, tag="stage")
                    for ti in range(tb):
                        ps = pps.tile([P, RW1], F32, tag="ps")
                        for kk in range(2):
                            nc.tensor.matmul(
                                ps[:], lhsT=xt[:, kk, ti * P:(ti + 1) * P],
                                rhs=w1_sb[:, kk, :],
                                start=(kk == 0), stop=(kk == 1))
                        if ti % 2 == 0:
                            nc.scalar.copy(stage[:, ti, :], ps[:])
                        else:
                            nc.vector.tensor_copy(stage[:, ti, :], ps[:])
                    # route tiles to the lo/hi tables
                    spans = []
                    if t0 < lo_tiles:
                        n_lo = min(tb, lo_tiles - t0)
                        spans.append((hx1_lo, t0 * P, 0, n_lo))
                        if n_lo < tb:
                            spans.append((hx1_hi, 0, n_lo, tb - n_lo))
                    else:
                        spans.append((hx1_hi, t0 * P - losplit, 0, tb))
                    for tab, r0, ti0, ntl in spans:
                        nc.sync.dma_start(
                            out=tab[r0:r0 + ntl * P, :RW1].rearrange(
                                "(t p) w -> p t w", p=P),
                            in_=stage[:, ti0:ti0 + ntl, :])
                    nc.sync.dma_start(
                        out=ad1[t0 * P:(t0 + tb) * P, :].rearrange(
                            "(t p) w -> p t w", p=P),
                        in_=stage[:, :tb, PAY1:PAY1 + H])
                # sentinel rows: full zero row with as=-30000 in lo; the hi
                # sentinel (a real pad row, h already 0) gets just as cols
                nc.sync.dma_start(out=hx1_lo[lo_sent:lo_sent + 1, :RW1],
                                  in_=sent_row[0:1, :])
                nc.sync.dma_start(
                    out=hx1_hi[hi_sent:hi_sent + 1, HC:HC + H],
                    in_=sent_row[0:1, HC:HC + H])

            # ---------------- phase B1: layer-1 aggregation + W2 fold ------
            with (
                tc.tile_pool(name="pb_gather", bufs=3) as gpool,
                tc.tile_pool(name="pb_work", bufs=3) as wpool,
                tc.tile_pool(name="pb_ep", bufs=2) as epool,
                tc.tile_pool(name="pb_psum", bufs=2, space="PSUM") as upps,
                tc.tile_pool(name="pb_psumA", bufs=2, space="PSUM") as apps,
                tc.tile_pool(name="pb_psumT", bufs=2, space="PSUM") as tpps,
            ):
                for g in range(ngroups):
                    pay = gpool.tile([P, ebt1, TRW1], F16, tag="pay")
                    nc.gpsimd.dma_gather(
                        out_ap=pay[:, :e1lo, :],
                        in_ap=hx1_lo[:],
                        idxs_ap=lo1_sb[:, g * e1lo * 8:(g + 1) * e1lo * 8],
                        num_idxs=e1lo * P, num_idxs_reg=e1lo * P,
                        elem_size=TRW1, single_packet=False)
                    nc.gpsimd.dma_gather(
                        out_ap=pay[:, e1lo:, :],
                        in_ap=hx1_hi[:],
                        idxs_ap=hi1_sb[:, g * e1hi * 8:(g + 1) * e1hi * 8],
                        num_idxs=e1hi * P, num_idxs_reg=e1hi * P,
                        elem_size=TRW1, single_packet=False)
                    adg = gpool.tile([P, H], F16, tag="adg")
                    nc.gpsimd.indirect_dma_start(
                        out=adg[:], out_offset=None, in_=ad1,
                        in_offset=bass.IndirectOffsetOnAxis(
                            ap=nodei[:, g:g + 1], axis=0))
                    cs1 = slice(g * ebt1, (g + 1) * ebt1)
                    msb = wpool.tile([P, ebt1, P], F16, tag="msb")
                    nc.vector.tensor_tensor(
                        out=msb[:],
                        in0=iota_f[:, None, :].to_broadcast([P, ebt1, P]),
                        in1=dstl1[:, cs1][:, :, None].to_broadcast(
                            [P, ebt1, P]),
                        op=mybir.AluOpType.is_equal)
                    msbT = wpool.tile([P, ebt1, P], F16, tag="msbT")
                    nc.sync.dma_start_transpose(out=msbT[:], in_=msb[:])
                    ad_ps = apps.tile([P, ebt1, H], F32, tag="ad_ps")
                    for j in range(ebt1):
                        nc.tensor.matmul(ad_ps[:, j, :], lhsT=msbT[:, j, :],
                                         rhs=adg[:], start=True, stop=True)
                    z = wpool.tile([P, ebt1, H], F32, tag="z")
                    nc.vector.tensor_tensor(
                        out=z[:], in0=pay[:, :, HC:HC + H], in1=ad_ps[:],
                        op=mybir.AluOpType.add)
                    z2 = wpool.tile([P, ebt1, H], F32, tag="z2")
                    nc.vector.tensor_scalar_mul(z2[:], z[:], 0.2)
                    lr = wpool.tile([P, ebt1, H], F32, tag="lr")
                    nc.vector.tensor_tensor(out=lr[:], in0=z[:], in1=z2[:],
                                            op=mybir.AluOpType.max)
                    gsb = wpool.tile([P, ebt1, PAY1], F16, tag="gsb")
                    nc.scalar.activation(
                        out=gsb[:, :, HC:], in_=lr[:],
                        func=mybir.ActivationFunctionType.Exp,
                        bias=nshift[:])
                    nc.vector.tensor_tensor(
                        out=gsb[:, :, :HC].rearrange(
                            "p j (c h) -> p j c h", h=H),
                        in0=pay[:, :, :HC].rearrange(
                            "p j (c h) -> p j c h", h=H),
                        in1=gsb[:, :, None, HC:].to_broadcast([P, ebt1, CH, H]),
                        op=mybir.AluOpType.mult)
                    u_ps = upps.tile([P, PAY1], F32, tag="u_ps")
                    for j in range(ebt1):
                        nc.tensor.matmul(u_ps[:], lhsT=msb[:, j, :],
                                         rhs=gsb[:, j, :],
                                         start=(j == 0), stop=(j == ebt1 - 1))
                    # epilogue
                    s_sb = epool.tile([P, H], F32, tag="s_sb")
                    nc.vector.tensor_scalar_add(s_sb[:], u_ps[:, HC:], 1e-16)
                    r_sb = epool.tile([P, H], F32, tag="r_sb")
                    nc.vector.reciprocal(r_sb[:], s_sb[:])
                    zt = epool.tile([P, HC], F32, tag="zt")
                    nc.vector.tensor_tensor(
                        out=zt[:].rearrange("p (c h) -> p c h", h=H),
                        in0=u_ps[:, :HC].rearrange("p (c h) -> p c h", h=H),
                        in1=r_sb[:][:, None, :].to_broadcast([P, CH, H]),
                        op=mybir.AluOpType.mult)
                    zb = epool.tile([P, HC], F16, tag="zb")
                    nc.vector.tensor_tensor(out=zb[:], in0=zt[:], in1=b1bc[:],
                                            op=mybir.AluOpType.add)
                    t1 = epool.tile([P, HC], F16, tag="t1")
                    nc.vector.tensor_scalar(out=t1[:], in0=zb[:], scalar1=0.0,
                                            scalar2=None,
                                            op0=mybir.AluOpType.min)
                    t2 = epool.tile([P, HC], F16, tag="t2")
                    nc.scalar.activation(out=t2[:], in_=t1[:],
                                         func=mybir.ActivationFunctionType.Exp)
                    t3 = epool.tile([P, HC], F16, tag="t3")
                    nc.vector.tensor_scalar_add(t3[:], t2[:], -1.0)
                    h16 = epool.tile([P, HC], F16, tag="h16")
                    nc.vector.tensor_tensor(out=h16[:], in0=zb[:], in1=t3[:],
                                            op=mybir.AluOpType.max)
                    h2_ps = tpps.tile([P, RW2], F32, tag="h2_ps")
                    for kk in range(2):
                        hT_ps = tpps.tile([P, P], F16, tag="hT_ps")
                        nc.tensor.transpose(hT_ps[:],
                                            h16[:, kk * P:(kk + 1) * P],
                                            ident16[:])
                        hT_sb = epool.tile([P, P], F16, tag="hT_sb")
                        nc.vector.tensor_copy(hT_sb[:], hT_ps[:])
                        nc.tensor.matmul(h2_ps[:], lhsT=hT_sb[:],
                                         rhs=w2_sb[:, kk, :],
                                         start=(kk == 0), stop=(kk == 1))
                    nc.scalar.copy(hx2_sb[:, g, :], h2_ps[:])
                    if split_cc and g == gh - 1:
                        # first-half allgather overlaps the remaining groups
                        nc.sync.dma_start(
                            out=cc_in[:gh * P].rearrange(
                                "(g p) w -> p g w", p=P),
                            in_=hx2_sb[:, :gh, :])
                        nc.gpsimd.collective_compute(
                            "AllGather", mybir.AluOpType.bypass,
                            replica_groups=[list(range(n_cores))],
                            ins=[cc_in[:gh * P]], outs=[hx2f_a[:]])
                if not split_cc:
                    nc.sync.dma_start(
                        out=cc_in[:gh * P].rearrange("(g p) w -> p g w", p=P),
                        in_=hx2_sb[:, :gh, :])
                    nc.gpsimd.collective_compute(
                        "AllGather", mybir.AluOpType.bypass,
                        replica_groups=[list(range(n_cores))],
                        ins=[cc_in[:gh * P]], outs=[hx2f_a[:]])
                nc.sync.dma_start(
                    out=cc_in[gh * P:].rearrange("(g p) w -> p g w", p=P),
                    in_=hx2_sb[:, gh:, :])
                # layer-2 sentinel: as2 = -30000 on the first pad row
                assert sent2 >= gh * P
                nc.sync.dma_start(
                    out=cc_in[sent2:sent2 + 1, PAY2 - 1:PAY2],
                    in_=sent_row[0:1, HC:HC + 1])
                nc.gpsimd.collective_compute(
                    "AllGather", mybir.AluOpType.bypass,
                    replica_groups=[list(range(n_cores))],
                    ins=[cc_in[gh * P:]], outs=[hx2f_b[:]])


            # ------------- expand hx2f into padded lo/hi tables ------------
            with tc.tile_pool(name="px", bufs=4) as xpool:
                for r in range(n_cores):
                    xt2 = xpool.tile([P, gh, RW2], F16, tag="xt2")
                    nc.sync.dma_start(
                        out=xt2[:],
                        in_=hx2f_a[r].rearrange("(g p) w -> p g w", p=P))
                    tab = hx2p_lo if r < halfc else hx2p_hi
                    r0 = (r % halfc) * npc_pad
                    nc.sync.dma_start(
                        out=tab[r0:r0 + gh * P, :PAY2 + 1].rearrange(
                            "(g p) w -> p g w", p=P),
                        in_=xt2[:, :, :PAY2 + 1])
                for r in range(n_cores):
                    xt3 = xpool.tile([P, ngroups - gh, RW2], F16, tag="xt3")
                    nc.sync.dma_start(
                        out=xt3[:],
                        in_=hx2f_b[r].rearrange("(g p) w -> p g w", p=P))
                    tab = hx2p_lo if r < halfc else hx2p_hi
                    r0 = (r % halfc) * npc_pad
                    nc.sync.dma_start(
                        out=tab[r0 + gh * P:r0 + npc_pad, :PAY2 + 1].rearrange(
                            "(g p) w -> p g w", p=P),
                        in_=xt3[:, :, :PAY2 + 1])
                if debug_outputs:
                    dt2 = xpool.tile([P, ngroups, RW2], F16, tag="dt2")
                    nc.sync.dma_start(
                        out=dt2[:],
                        in_=cc_in[:].rearrange("(g p) w -> p g w", p=P))
                    nc.sync.dma_start(
                        out=dbg_cc[:].rearrange("(g p) w -> p g w", p=P),
                        in_=dt2[:])

            # ---------------- phase B2: layer-2 aggregation ----------------
            with (
                tc.tile_pool(name="p2_gather", bufs=3) as g2pool,
                tc.tile_pool(name="p2_work", bufs=3) as w2pool,
                tc.tile_pool(name="p2_ep", bufs=2) as e2pool,
                tc.tile_pool(name="p2_psum", bufs=2, space="PSUM") as u2ps,
                tc.tile_pool(name="p2_psumA", bufs=2, space="PSUM") as a2ps,
            ):
                for g in range(ngroups):
                    pay = g2pool.tile([P, ebt2, TRW2], F16, tag="pay2")
                    nc.gpsimd.dma_gather(
                        out_ap=pay[:, :e2lo, :],
                        in_ap=hx2p_lo[:],
                        idxs_ap=lo2_sb[:, g * e2lo * 8:(g + 1) * e2lo * 8],
                        num_idxs=e2lo * P, num_idxs_reg=e2lo * P,
                        elem_size=TRW2, single_packet=False)
                    nc.gpsimd.dma_gather(
                        out_ap=pay[:, e2lo:, :],
                        in_ap=hx2p_hi[:],
                        idxs_ap=hi2_sb[:, g * e2hi * 8:(g + 1) * e2hi * 8],
                        num_idxs=e2hi * P, num_idxs_reg=e2hi * P,
                        elem_size=TRW2, single_packet=False)
                    cs2 = slice(g * ebt2, (g + 1) * ebt2)
                    msb = w2pool.tile([P, ebt2, P], F16, tag="msb2")
                    nc.vector.tensor_tensor(
                        out=msb[:],
                        in0=iota_f[:, None, :].to_broadcast([P, ebt2, P]),
                        in1=dstl2[:, cs2][:, :, None].to_broadcast(
                            [P, ebt2, P]),
                        op=mybir.AluOpType.is_equal)
                    msbT = w2pool.tile([P, ebt2, P], F16, tag="msbT2")
                    nc.sync.dma_start_transpose(out=msbT[:], in_=msb[:])
                    ad_ps = a2ps.tile([P, ebt2, 1], F32, tag="ad_ps2")
                    for j in range(ebt2):
                        nc.tensor.matmul(ad_ps[:, j, :], lhsT=msbT[:, j, :],
                                         rhs=hx2_sb[:, g, PAY2:PAY2 + 1],
                                         start=True, stop=True)
                    z = w2pool.tile([P, ebt2, 1], F32, tag="z")
                    nc.vector.tensor_tensor(out=z[:],
                                            in0=pay[:, :, OUT:OUT + 1],
                                            in1=ad_ps[:],
                                            op=mybir.AluOpType.add)
                    z2 = w2pool.tile([P, ebt2, 1], F32, tag="z2")
                    nc.vector.tensor_scalar_mul(z2[:], z[:], 0.2)
                    lr = w2pool.tile([P, ebt2, 1], F32, tag="lr")
                    nc.vector.tensor_tensor(out=lr[:], in0=z[:], in1=z2[:],
                                            op=mybir.AluOpType.max)
                    gsb = w2pool.tile([P, ebt2, PAY2], F16, tag="gsb2")
                    nc.scalar.activation(
                        out=gsb[:, :, OUT:], in_=lr[:],
                        func=mybir.ActivationFunctionType.Exp, bias=0.0)
                    nc.vector.tensor_tensor(
                        out=gsb[:, :, :OUT],
                        in0=pay[:, :, :OUT],
                        in1=gsb[:, :, OUT:].to_broadcast([P, ebt2, OUT]),
                        op=mybir.AluOpType.mult)
                    u_ps = u2ps.tile([P, PAY2], F32, tag="u_ps2")
                    for j in range(ebt2):
                        nc.tensor.matmul(u_ps[:], lhsT=msb[:, j, :],
                                         rhs=gsb[:, j, :],
                                         start=(j == 0), stop=(j == ebt2 - 1))
                    s_sb = e2pool.tile([P, 1], F32, tag="s_sb2")
                    nc.vector.tensor_scalar_add(s_sb[:], u_ps[:, OUT:], 1e-16)
                    r_sb = e2pool.tile([P, 1], F32, tag="r_sb2")
                    nc.vector.reciprocal(r_sb[:], s_sb[:])
                    y_sb = e2pool.tile([P, OUT], F32, tag="y_sb")
                    nc.scalar.activation(
                        out=y_sb[:], in_=u_ps[:, :OUT],
                        func=mybir.ActivationFunctionType.Copy,
                        scale=r_sb[:, 0:1])
                    yb = e2pool.tile([P, OUT], F32, tag="yb")
                    nc.vector.tensor_tensor(out=yb[:], in0=y_sb[:],
                                            in1=b2bc[:],
                                            op=mybir.AluOpType.add)
                    nc.sync.dma_start(out=y_ap[g * P:(g + 1) * P, :],
                                      in_=yb[:])
    nc.compile()
    return nc


# --------------------------------------------------------------------------
# host-side input prep
# --------------------------------------------------------------------------

def prep_inputs(inputs, cfg, tables):
    x = np.asarray(inputs["x"], np.float32)
    npad1 = cfg["npad1"]
    xT = np.zeros((D, npad1), np.float16)
    xT[:, :cfg["n_valid"]] = x.T.astype(np.float16)
    xT16 = np.ascontiguousarray(xT.reshape(2, P, npad1))
    W1ext, W2ext = fold_weights(
        inputs["W1"], inputs["a_src1"], inputs["a_dst1"],
        inputs["W2"], inputs["a_src2"], inputs["a_dst2"])
    b1 = np.asarray(inputs["b1"], np.float32)
    b2 = np.asarray(inputs["b2"], np.float32)
    in_maps = [dict(
        xT16=xT16, w1ext=W1ext, b1=b1, w2ext=W2ext, b2=b2,
        lo1=t["lo1"], hi1=t["hi1"], dstl1=t["dstl1"],
        lo2=t["lo2"], hi2=t["hi2"], dstl2=t["dstl2"],
        nodeidx=t["nodeidx"],
    ) for t in tables]
    return in_maps


_CACHE = {}


import os as _os


def get_nc(cfg, ebs):
    split = _os.environ.get("K_SPLIT_CC", "1") == "1"
    key = (cfg["n_valid"], cfg["n_cores"], ebs, split)
    if key not in _CACHE:
        _CACHE[key] = build_fused(cfg, ebs, split_cc=split)
    return _CACHE[key]


def _run_with_retry(nc, in_maps, tries=3):
    from concourse.bass_utils import run_bass_kernel_spmd
    last = None
    for attempt in range(tries):
        try:
            return run_bass_kernel_spmd(nc, in_maps,
                                        core_ids=list(range(len(in_maps))))
        except Exception as e:  # noqa: BLE001 - retry any runtime failure
            last = e
            import time as _time
            _time.sleep(2.0 * (attempt + 1))
    raise last


def kernel(**inputs):
    """Full-input GAT kernel on 8 Trainium2 NeuronCores.

    Takes the unsharded inputs of reference.setup_inputs(), distributes the
    work across 8 cores (dst-node graph partition) in a single fused launch
    with an on-device AllGather between the layers, and returns the full
    [50000, 32] float32 output.
    """
    x = np.asarray(inputs["x"], np.float32)
    ei = np.asarray(inputs["edge_index"])
    N = x.shape[0]
    cfg = make_cfg(N)
    src = np.concatenate([ei[0].astype(np.int64),
                          np.arange(N, dtype=np.int64)])
    dst = np.concatenate([ei[1].astype(np.int64),
                          np.arange(N, dtype=np.int64)])
    tables, ebs = build_edge_tables(src, dst, cfg)
    nc = get_nc(cfg, ebs)
    in_maps = prep_inputs(inputs, cfg, tables)
    res = _run_with_retry(nc, in_maps)
    npc = cfg["npc"]
    y = np.concatenate([res.results[k]["y"][:npc]
                        for k in range(cfg["n_cores"])], axis=0)
    return y.astype(np.float32)
